# revision 28
# baseline (speedup 1.0000x reference)
"""Trainium2 Bass kernel for nn_CMmodel (retrieval_knn).

Model (per layer, x2):
    sim = cosine(x, mem)                       # [N, 2048]
    S, I = top_k(sim, 10); w = softmax(relu(S))
    h = sum_k w[n,k] * mem[I[n,k]]             # [N, 256]
    h = leaky_relu(batchnorm(h))               # batch stats over ALL N rows

Strategy (8 cores, data-parallel over N):
  - Shard x rows 8 ways; replicate mem banks + BN params.
  - sim via PE matmul, 3-pass f32r exactness scheme (top-10 selection is
    precision-critical: a 10/11 rank swap is a ~50% error on that row, and
    even 1e-6 sim noise swaps ~30 rows of 32k).
  - Exact top-10 threshold t via DVE max8 + match_replace + max8.
  - U = (s>=t)*exp(s-t) via fused DVE scalar_tensor_tensor w/ accum Z.
  - h = U @ mem via PE: U transposed 128x128 on PE (fp32 for layer 1 since
    h1 feeds layer-2 sims; f32r for layer 2), matmul vs raw mem.
  - BatchNorm batch stats via ones-matmul into PSUM, deferred one tile so
    the PE never waits on the ACT/DVE drain chain; AllReduce'd (2KB).
  - ALL activations live in the 'natural_log_exp_and_others' ACT table set
    (Exp, Ln, Prelu, Copy, Square): rsqrt = exp(-0.5*ln), leaky = Prelu.
    Zero table swaps in steady state.
  - mem2 bank prep is emitted after layer 1 so it fills the AllReduce
    bubble; h2 stays SBUF-resident for the final BN2 apply.
"""
import sys

sys.path.insert(0, "/opt/trn_rl_repo")

import numpy as np

import concourse.bacc as bacc
import concourse.mybir as mybir
import concourse.tile as tile
from concourse.bass_utils import run_bass_kernel_spmd
from concourse.masks import make_identity
from concourse.tile import add_dep_helper

F32 = mybir.dt.float32
F32R = mybir.dt.float32r
BF16 = mybir.dt.bfloat16
AF = mybir.ActivationFunctionType
OP = mybir.AluOpType

MEM_DIM = 256
MEM_SIZE = 2048
K_TOP = 10
BN_EPS = 1e-5
LEAKY = 0.01

NJ = MEM_SIZE // 128  # 16 mem-row chunks
NEG_BIG = -1e30


def build_nc(n_cores: int, rows_per_core: int, use_f32r: bool = True):
    """Build the per-core Bass program (SPMD: same program all cores)."""
    nt = rows_per_core // 128  # x tiles per core
    n_total = rows_per_core * n_cores
    MMDT = F32R if use_f32r else F32
    nc = bacc.Bacc("TRN2", target_bir_lowering=False, debug=False,
                   num_devices=n_cores)

    x_d = nc.dram_tensor("x", [rows_per_core, MEM_DIM], F32, kind="ExternalInput")
    mem_d = {
        1: nc.dram_tensor("mem1", [MEM_SIZE, MEM_DIM], F32, kind="ExternalInput"),
        2: nc.dram_tensor("mem2", [MEM_SIZE, MEM_DIM], F32, kind="ExternalInput"),
    }
    gam_d = {
        1: nc.dram_tensor("gamma1", [1, MEM_DIM], F32, kind="ExternalInput"),
        2: nc.dram_tensor("gamma2", [1, MEM_DIM], F32, kind="ExternalInput"),
    }
    bet_d = {
        1: nc.dram_tensor("beta1", [1, MEM_DIM], F32, kind="ExternalInput"),
        2: nc.dram_tensor("beta2", [1, MEM_DIM], F32, kind="ExternalInput"),
    }
    out_d = nc.dram_tensor("out", [rows_per_core, MEM_DIM], F32, kind="ExternalOutput")

    with tile.TileContext(nc) as tc:
        with tc.tile_pool(name="consts", bufs=1) as consts, \
             tc.tile_pool(name="banks", bufs=1) as banks, \
             tc.tile_pool(name="work", bufs=1) as work, \
             tc.tile_pool(name="psum_sim", bufs=3, space="PSUM") as psum_sim, \
             tc.tile_pool(name="psum_tp", bufs=2, space="PSUM") as psum_tp, \
             tc.tile_pool(name="psum_h", bufs=2, space="PSUM") as psum_h_pool, \
             tc.tile_pool(name="psum_st", bufs=1, space="PSUM") as psum_st, \
             tc.tile_pool(name="dram", bufs=1, space="DRAM") as dram:

            # PE emission-order chain: accumulation groups must stay
            # contiguous on PE (interleaved matmuls drop accumulates).
            class _PEChain:
                def __init__(self):
                    self.last = None

                def _chain(self, binst):
                    if self.last is not None:
                        add_dep_helper(binst.ins, self.last.ins, sync=False,
                                       reason="pe-order")
                    self.last = binst
                    return binst

                def matmul(self, *a, **kw):
                    return self._chain(nc.tensor.matmul(*a, **kw))

                def transpose(self, *a, **kw):
                    return self._chain(nc.tensor.transpose(*a, **kw))

            PE = _PEChain()

            # ---------------- constants ----------------
            ident = consts.tile([128, 128], F32)
            make_identity(nc, ident)
            ident_b = consts.tile([128, 128], BF16)
            nc.scalar.copy(ident_b, ident)  # exact: 0/1 values
            ones_col = consts.tile([128, 1], F32)
            nc.vector.memset(ones_col, 1.0)
            ones_col_b = consts.tile([128, 1], BF16)
            nc.vector.memset(ones_col_b, 1.0)
            one_1x1 = consts.tile([1, 1], F32)
            nc.vector.memset(one_1x1, 1.0)
            ones_row = consts.tile([1, 128], F32)
            nc.vector.memset(ones_row, 1.0)
            epsap = consts.tile([1, 1], F32)
            nc.vector.memset(epsap, BN_EPS)

            gb = {}
            for L in (1, 2):
                g = consts.tile([1, MEM_DIM], F32, name=f"gamma_sb{L}")
                b = consts.tile([1, MEM_DIM], F32, name=f"beta_sb{L}")
                nc.sync.dma_start(g, gam_d[L][:])
                nc.sync.dma_start(b, bet_d[L][:])
                gb[L] = (g, b)

            # ---------------- mem banks (prep emitted lazily) ----------------
            # mraw_b[L]: raw mem, natural layout [128, NJ*256] (rhs of h mm)
            # mnT[L,k] : row-normalized mem, transposed, f32r-rounded
            # mnTres   : bf16 residual (m/||m|| - round(m/||m||))
            mraw_b = {}
            mnT = {}
            mnTres = {}
            for L in (1, 2):
                # L1 h-matmul must be fp32-exact (h1 feeds layer-2 sims);
                # L2's only feeds the final output, so bf16 is plenty.
                mraw_b[L] = banks.tile([128, NJ * MEM_DIM],
                                       F32 if L == 1 else BF16, name=f"mraw{L}")
                mnT[L] = [
                    banks.tile([128, MEM_SIZE], MMDT, name=f"mnT{L}_{k}")
                    for k in range(2)
                ]
                mnTres[L] = [
                    banks.tile([128, MEM_SIZE], BF16, name=f"mnTres{L}_{k}")
                    for k in range(2)
                ]

            def prep_bank(L):
                """Load + normalize + transpose one memory bank.

                Batch the norm chain across all 16 chunks (one Ln+Exp rsqrt
                + one batched Newton refine), then per-chunk scale + PE
                transposes. L1 DMAs straight into its fp32 bank; L2's bank
                is bf16, so the fp32 rows are staged (DMA'd twice - once
                for norms, once for scale+convert) through work tiles.
                """
                def stage_in(j, bufs=2):
                    if L == 1:
                        return mraw_b[1][:, j * MEM_DIM:(j + 1) * MEM_DIM]
                    stg = work.tile([128, MEM_DIM], F32, tag="mstg",
                                    name="mstg", bufs=bufs)
                    nc.sync.dma_start(stg, mem_d[L][j * 128:(j + 1) * 128, :])
                    return stg

                if L == 1:
                    for j in range(NJ):
                        nc.sync.dma_start(
                            mraw_b[1][:, j * MEM_DIM:(j + 1) * MEM_DIM],
                            mem_d[1][j * 128:(j + 1) * 128, :])
                mns = work.tile([128, NJ], F32, tag="mns", name="mns", bufs=1)
                for j in range(NJ):
                    src = stage_in(j)
                    msq = work.tile([128, MEM_DIM], F32, tag="sqs", name="sqs",
                                    bufs=2)
                    nc.scalar.activation(msq, src, AF.Square,
                                         accum_out=mns[:, j:j + 1])
                # batched rsqrt, all on DVE (mem-norm precision reorders
                # near-tied sims; 2 Newton steps make it fp32-exact)
                inm = work.tile([128, NJ], F32, tag="inm", name="inm", bufs=1)
                rsqrt_dve(inm, mns, iters=4, seed=1.73)
                for j in range(NJ):
                    src = stage_in(j)
                    if L == 2:  # bf16 bank copy for the h2 matmul rhs
                        nc.vector.tensor_copy(
                            mraw_b[2][:, j * MEM_DIM:(j + 1) * MEM_DIM], src)
                    mnsc = work.tile([128, MEM_DIM], F32, tag="mnsc", name="mnsc",
                                     bufs=2)
                    nc.scalar.mul(mnsc, src, inm[:, j:j + 1])
                    for k in range(2):
                        tp = psum_tp.tile([128, 128], F32, tag="tp2")
                        PE.transpose(tp, mnsc[:, k * 128:(k + 1) * 128], ident)
                        dstT = mnT[L][k][:, j * 128:(j + 1) * 128]
                        nc.scalar.copy(dstT, tp)                   # f32r round
                        # bf16 residual straight from psum - rounded
                        nc.vector.tensor_sub(
                            mnTres[L][k][:, j * 128:(j + 1) * 128],
                            tp, dstT.bitcast(F32))

            # ---------------- persistent stores ----------------
            # h1 and h2 both spill to DRAM (SBUF goes to pipeline buffers)
            h1_dram = nc.dram_tensor("h1buf", [rows_per_core, MEM_DIM], F32)
            h2_dram = nc.dram_tensor("h2buf", [rows_per_core, MEM_DIM], BF16)
            # BN affine params (filled after each AllReduce)
            aT = [consts.tile([128, 1], F32, name=f"aT{k}") for k in range(2)]
            bT = [consts.tile([128, 1], F32, name=f"bT{k}") for k in range(2)]
            a2b = consts.tile([128, MEM_DIM], F32, name="a2b")
            b2b = consts.tile([128, MEM_DIM], F32, name="b2b")

            def rsqrt_dve(dst, src, iters=3, seed="recip", gp=False):
                """dst = 1/sqrt(src), entirely on DVE, keeping Sqrt off the
                ACT engine so the Exp table never gets evicted (each table
                load costs 1.3us). Seed: either a float constant, or
                "recip" = secant fit a*(1/x)+b of sqrt(1/x) over x in
                [60, 400] (max 10% off). Newton steps y*(1.5-0.5*x*y^2)
                converge regardless: 3 iters from 10% -> ~2e-7 rel."""
                shp = list(src.shape)
                eng = nc.gpsimd if gp else nc.vector
                y = work.tile(shp, F32, tag="ny0", name="ny0", bufs=2)
                if seed == "recip":
                    rr = work.tile(shp, F32, tag="nrr", name="nrr", bufs=2)
                    nc.vector.reciprocal(rr, src)
                    eng.tensor_scalar(y, rr, 5.56, 0.0361,
                                      op0=OP.mult, op1=OP.add)
                else:
                    eng.tensor_scalar(y, src, 0.0, float(seed),
                                      op0=OP.mult, op1=OP.add)
                for it in range(iters):
                    t1 = work.tile(shp, F32, tag="nt2", name="nt2", bufs=2)
                    eng.tensor_mul(t1, y, y)
                    eng.tensor_mul(t1, t1, src)
                    eng.tensor_scalar(t1, t1, -0.5, 1.5, op0=OP.mult,
                                      op1=OP.add)
                    out = dst if it == iters - 1 else work.tile(
                        shp, F32, tag=f"ny{it + 1}", name=f"ny{it + 1}", bufs=2)
                    eng.tensor_mul(out, y, t1)
                    y = out

            def stage1a(L, i):
                """lhsT prep for tile i: DMA + transposes + f32r/bf16 splits
                + row-norm rsqrt. Emitted one tile AHEAD of stage1b(i-1)'s
                sims so the ACT queue serves these copies before that tile's
                exp/uts work (otherwise the PE stalls on LDWEIGHTS).
                Small elementwise goes to the idle GpSimd engine."""
                lhsT_f = [
                    work.tile([128, 128], F32, tag=f"lhsTf{k}", name=f"lhsTf{k}", bufs=2)
                    for k in range(2)
                ]
                lhsT_r = [
                    work.tile([128, 128], MMDT, tag=f"lhsTr{k}", name=f"lhsTr{k}", bufs=2)
                    for k in range(2)
                ]
                lhsT_s = [
                    work.tile([128, 128], MMDT, tag=f"lhsTs{k}", name=f"lhsTs{k}", bufs=2)
                    for k in range(2)
                ]
                lhsT_rb = [
                    work.tile([128, 128], BF16, tag=f"lhsTb{k}", name=f"lhsTb{k}", bufs=2)
                    for k in range(2)
                ]
                if L == 1:
                    invn = invn_all[:, i:i + 1]
                    xi = work.tile([128, MEM_DIM], F32, tag="xi", name="xi", bufs=3)
                    nc.sync.dma_start(xi, x_d[i * 128:(i + 1) * 128, :])
                    tpx = psum_tp.tile([128, 256], F32, tag="tp2")
                    for k in range(2):
                        PE.transpose(tpx[:, k * 128:(k + 1) * 128],
                                     xi[:, k * 128:(k + 1) * 128], ident)
                    for k in range(2):
                        nc.scalar.copy(lhsT_f[k], tpx[:, k * 128:(k + 1) * 128])
                else:
                    ns_ps = psum_tp.tile([1, 128], F32, tag="tp2")
                    hsl = work.tile([128, MEM_DIM], F32, tag="h1i", name="h1i", bufs=3)
                    nc.sync.dma_start(hsl, h1_dram[i * 128:(i + 1) * 128, :])
                    sqTs = []
                    tph = psum_tp.tile([128, 256], F32, tag="tp2")
                    for k in range(2):
                        PE.transpose(tph[:, k * 128:(k + 1) * 128],
                                     hsl[:, k * 128:(k + 1) * 128], ident)
                    for k in range(2):
                        # fused BN1 apply + leaky relu at the drain (Prelu
                        # shares the Exp ACT table set; Lrelu doesn't)
                        nc.scalar.activation(
                            lhsT_f[k], tph[:, k * 128:(k + 1) * 128], AF.Prelu,
                            bias=bT[k], scale=aT[k], alpha=LEAKY,
                        )
                        sqT = work.tile([128, 128], F32, tag=f"sqT{k}", name=f"sqT{k}", bufs=2)
                        nc.vector.tensor_mul(sqT, lhsT_f[k], lhsT_f[k])
                        sqTs.append(sqT)
                    for k in range(2):
                        PE.matmul(ns_ps, ones_col, sqTs[k],
                                  start=(k == 0), stop=(k == 1))
                    ns_sb = work.tile([1, 128], F32, tag="ns_sb", name="ns_sb", bufs=2)
                    nc.scalar.copy(ns_sb, ns_ps)
                    nsT = psum_tp.tile([128, 1], F32, tag="tp2")
                    PE.transpose(nsT, ns_sb, one_1x1)
                    invn = work.tile([128, 1], F32, tag="invn", name="invn",
                                     bufs=2)
                    nsS = work.tile([128, 1], F32, tag="nsS", name="nsS", bufs=2)
                    nc.scalar.copy(nsS, nsT)
                    # Newton on ACT (Square/Copy only - table-safe); just
                    # the reciprocal seed runs on DVE. Keeps the L2 DVE
                    # queue clear for the topk chain.
                    rr = work.tile([128, 1], F32, tag="nrr", name="nrr", bufs=2)
                    nc.vector.reciprocal(rr, nsS)
                    y = work.tile([128, 1], F32, tag="ny0", name="ny0", bufs=2)
                    nc.scalar.activation(y, rr, AF.Copy, bias=0.0361, scale=5.56)
                    for it in range(3):
                        t1 = work.tile([128, 1], F32, tag="nt2", name="nt2",
                                       bufs=2)
                        nc.scalar.activation(t1, y, AF.Square)
                        nc.scalar.mul(t1, t1, nsS)
                        nc.scalar.activation(t1, t1, AF.Copy, bias=1.5,
                                             scale=-0.5)
                        yn = invn if it == 2 else work.tile(
                            [128, 1], F32, tag=f"ny{it + 1}",
                            name=f"ny{it + 1}", bufs=2)
                        nc.scalar.mul(yn, t1, y)
                        y = yn
                for k in range(2):
                    nc.scalar.copy(lhsT_r[k], lhsT_f[k])   # f32r round
                    nc.vector.tensor_copy(lhsT_rb[k], lhsT_f[k])  # bf16 (pass C)
                    rsd = work.tile([128, 128], F32, tag="rsd", name="rsd", bufs=2)
                    nc.vector.tensor_sub(rsd, lhsT_f[k], lhsT_r[k].bitcast(F32))
                    nc.scalar.copy(lhsT_s[k], rsd)         # f32r residual
                return dict(lhsT_r=lhsT_r, lhsT_s=lhsT_s, lhsT_rb=lhsT_rb,
                            invn=invn)

            def stage1b(L, i, sa):
                """3-pass sim matmul + top-10 threshold + masked exp
                weights for tile i (lhsT prepped by stage1a)."""
                lhsT_r, lhsT_s = sa["lhsT_r"], sa["lhsT_s"]
                lhsT_rb, invn = sa["lhsT_rb"], sa["invn"]
                # 3-pass f32r sim matmul: r(x)@r(m) + res_x@r(m) + x_b@res_m_b
                s_sb = work.tile([128, MEM_SIZE], F32, tag="s_sb", name="s_sb", bufs=2)
                for c in range(4):
                    ps = psum_sim.tile([128, 512], F32, tag="sim")
                    for k in range(2):
                        PE.matmul(ps, lhsT_r[k],
                                  mnT[L][k][:, c * 512:(c + 1) * 512],
                                  start=(k == 0), stop=False)
                    for k in range(2):
                        PE.matmul(ps, lhsT_s[k],
                                  mnT[L][k][:, c * 512:(c + 1) * 512],
                                  start=False, stop=False)
                    for k in range(2):
                        PE.matmul(ps, lhsT_rb[k],
                                  mnTres[L][k][:, c * 512:(c + 1) * 512],
                                  start=False, stop=(k == 1))
                    nc.scalar.mul(s_sb[:, c * 512:(c + 1) * 512], ps, invn)

                # exact 10th-largest threshold
                m8a = work.tile([128, 8], F32, tag="m8a", name="m8a", bufs=2)
                nc.vector.max(out=m8a, in_=s_sb)
                s_z = work.tile([128, MEM_SIZE], F32, tag="s_z", name="s_z", bufs=2)
                nc.vector.match_replace(out=s_z, in_to_replace=m8a,
                                        in_values=s_sb, imm_value=NEG_BIG)
                m8b = work.tile([128, 8], F32, tag="m8b", name="m8b", bufs=2)
                nc.vector.max(out=m8b, in_=s_z)
                t_ap = m8b[:, K_TOP - 8 - 1:K_TOP - 8]  # 10th largest
                neg_t = work.tile([128, 1], F32, tag="neg_t", name="neg_t", bufs=2)
                nc.vector.tensor_scalar(neg_t, t_ap, -1.0, None, op0=OP.mult)

                # shifted exp weights, masked, with sum
                e = work.tile([128, MEM_SIZE], F32, tag="e", name="e", bufs=1)
                nc.scalar.activation(e, s_sb, AF.Exp, bias=neg_t, scale=1.0)
                if L == 1:
                    U = s_z  # reuse buffer: s_z is dead after m8b
                else:
                    # bf16 weights: DVE rounds on write, so the PE's bf16
                    # fast path (1 cy/row transposes + h2 matmul) is legal
                    U = work.tile([128, MEM_SIZE], BF16, tag="Ub", name="Ub",
                                  bufs=2)
                Z = work.tile([128, 1], F32, tag="Z", name="Z", bufs=2)
                nc.vector.scalar_tensor_tensor(
                    out=U, in0=s_sb, scalar=t_ap, in1=e,
                    op0=OP.is_ge, op1=OP.mult, accum_out=Z,
                )
                rz = work.tile([128, 1], F32, tag="rz", name="rz", bufs=2)
                nc.vector.reciprocal(rz, Z)
                return dict(U=U, rz=rz)

            def stage2(L, i, st, stats_acc, pend):
                """U transposes + h = (U/Z) @ mem + BN batch-stat partials.

                The stats matmuls for THIS tile are deferred: they're pushed
                on `pend` and emitted by the NEXT stage2 call (or the flush),
                so the PE never waits on the dst/sqh drain chain.
                """
                U, rz = st["U"], st["rz"]
                # layer 1 feeds layer-2 sims: must stay fp32-exact.
                # layer 2 only feeds the final output: bf16 is plenty and
                # transposes at 1 cy/row instead of fp32's 2.5.
                ut_dt = F32 if L == 1 else BF16
                tp_ident = ident if L == 1 else ident_b
                # paired transposes -> one [128,256] drain per pair
                uts = []
                for c2 in range(NJ // 2):
                    tp2 = psum_tp.tile([128, 256], ut_dt, tag="tp2")
                    PE.transpose(tp2[:, 0:128],
                                 U[:, (2 * c2) * 128:(2 * c2 + 1) * 128], tp_ident)
                    PE.transpose(tp2[:, 128:256],
                                 U[:, (2 * c2 + 1) * 128:(2 * c2 + 2) * 128], tp_ident)
                    utp = work.tile([128, 256], ut_dt, tag="ut", name="ut",
                                    bufs=NJ // 2 + 2)
                    nc.scalar.copy(utp, tp2)
                    uts.append(utp)
                hp = psum_h_pool.tile([128, MEM_DIM], F32, tag="hp")
                for c in range(NJ):
                    PE.matmul(
                        hp, uts[c // 2][:, (c % 2) * 128:(c % 2 + 1) * 128],
                        mraw_b[L][:, c * MEM_DIM:(c + 1) * MEM_DIM],
                        start=(c == 0), stop=(c == NJ - 1),
                    )
                # drain h (normalized by Z) + square; stats matmuls deferred.
                # h2 is drained + spilled + stat'd in bf16 (halves the tail
                # DMA; BN2 normalizes the rounded values consistently).
                dst = work.tile([128, MEM_DIM], F32 if L == 1 else BF16,
                                tag="h2o", name="h2o", bufs=4)
                nc.scalar.mul(dst, hp, rz)
                h_dram = h1_dram if L == 1 else h2_dram
                nc.sync.dma_start(h_dram[i * 128:(i + 1) * 128, :], dst)
                sqh = work.tile([128, MEM_DIM], F32, tag="sqh", name="sqh", bufs=3)
                nc.vector.tensor_mul(sqh, dst, dst)
                emit_pending_stats(L, stats_acc, pend)
                pend.append((dst, sqh))

            def emit_pending_stats(L, stats_acc, pend):
                ones_l = ones_col if L == 1 else ones_col_b
                while pend:
                    dst, sqh = pend.pop(0)
                    pd = psum_st.tile([1, 512], F32, tag="st")
                    PE.matmul(pd[:, 0:MEM_DIM], ones_l, dst,
                              start=True, stop=True)
                    PE.matmul(pd[:, MEM_DIM:2 * MEM_DIM], ones_col, sqh,
                              start=True, stop=True)
                    nc.vector.tensor_add(stats_acc, stats_acc, pd)

            def layer(L):
                stats_acc = work.tile([1, 512], F32, tag=f"stacc{L}", bufs=1,
                                      name=f"stats_acc{L}")
                nc.vector.memset(stats_acc, 0.0)
                pend = []
                prev = None
                sa = stage1a(L, 0)
                for i in range(nt):
                    sa_next = stage1a(L, i + 1) if i + 1 < nt else None
                    st = stage1b(L, i, sa)
                    sa = sa_next
                    if prev is not None:
                        stage2(L, i - 1, prev, stats_acc, pend)
                    prev = st
                stage2(L, nt - 1, prev, stats_acc, pend)
                emit_pending_stats(L, stats_acc, pend)
                return stats_acc

            def bn_allreduce(L, stats_acc):
                gamma_sb, beta_sb = gb[L]
                stats_sb = stats_acc
                ar_in = dram.tile([1, 512], F32, name=f"ar_in{L}")
                ar_out = dram.tile([1, 512], F32, addr_space="Shared",
                                   name=f"ar_out{L}")
                nc.sync.dma_start(ar_in, stats_sb)
                nc.gpsimd.collective_compute(
                    "AllReduce", OP.add,
                    replica_groups=[list(range(n_cores))],
                    ins=[ar_in[:]], outs=[ar_out[:]],
                )
                gst = work.tile([1, 512], F32, tag="gst", name="gst", bufs=1)
                nc.sync.dma_start(gst, ar_out)

                ab = work.tile([1, 512], F32, tag="ab", name="ab", bufs=1)
                a_ap, b_ap = ab[:, 0:MEM_DIM], ab[:, MEM_DIM:512]
                mu = work.tile([1, MEM_DIM], F32, tag="mu", name="mu", bufs=1)
                nc.vector.tensor_scalar(mu, gst[:, 0:MEM_DIM], 1.0 / n_total,
                                        None, op0=OP.mult)
                ex2 = work.tile([1, MEM_DIM], F32, tag="ex2", name="ex2", bufs=1)
                nc.vector.tensor_scalar(ex2, gst[:, MEM_DIM:512], 1.0 / n_total,
                                        None, op0=OP.mult)
                musq = work.tile([1, MEM_DIM], F32, tag="musq", name="musq", bufs=1)
                nc.vector.tensor_mul(musq, mu, mu)
                var = work.tile([1, MEM_DIM], F32, tag="var", name="var", bufs=1)
                nc.vector.tensor_sub(var, ex2, musq)
                sd = work.tile([1, MEM_DIM], F32, tag="sd", name="sd", bufs=1)
                nc.scalar.activation(sd, var, AF.Sqrt, bias=epsap)
                isd = work.tile([1, MEM_DIM], F32, tag="isd", name="isd", bufs=1)
                nc.vector.reciprocal(isd, sd)
                nc.vector.tensor_mul(a_ap, gamma_sb, isd)
                mua = work.tile([1, MEM_DIM], F32, tag="mua", name="mua", bufs=1)
                nc.vector.tensor_mul(mua, mu, a_ap)
                nc.vector.tensor_sub(b_ap, beta_sb, mua)

                if L == 1:
                    # per-partition (transposed-layout) affine params
                    for k in range(2):
                        for src, dstp in ((a_ap, aT[k]), (b_ap, bT[k])):
                            tp = psum_tp.tile([128, 1], F32, tag="tp2")
                            PE.transpose(
                                tp, src[:, k * 128:(k + 1) * 128], one_1x1)
                            nc.scalar.copy(dstp, tp)
                else:
                    # broadcast across partitions (row-layout affine)
                    bc = psum_sim.tile([128, 512], F32, tag="sim")
                    PE.matmul(bc, ones_row, ab, start=True, stop=True)
                    nc.scalar.copy(a2b, bc[:, 0:MEM_DIM])
                    nc.scalar.copy(b2b, bc[:, MEM_DIM:512])

            # L1 row-norm prologue: second read of x during the prep
            # window computes every tile's 1/||x|| in one batched chain,
            # so the steady-state DVE queue holds only topk+weights work.
            invn_all = consts.tile([128, nt], F32, name="invn_all")
            xns_all = consts.tile([128, nt], F32, name="xns_all")
            prep_bank(1)
            for i in range(nt):
                xpr = work.tile([128, MEM_DIM], F32, tag="xpr", name="xpr",
                                bufs=3)
                nc.sync.dma_start(xpr, x_d[i * 128:(i + 1) * 128, :])
                xsq = work.tile([128, MEM_DIM], F32, tag="sqs", name="sqs",
                                bufs=2)
                nc.vector.scalar_tensor_tensor(
                    out=xsq, in0=xpr, scalar=0.0, in1=xpr,
                    op0=OP.add, op1=OP.mult, accum_out=xns_all[:, i:i + 1])
            rsqrt_dve(invn_all, xns_all, iters=3, seed="recip")
            s1 = layer(1)
            prep_bank(2)  # fills the AllReduce bubble + L1 tail
            bn_allreduce(1, s1)
            bn_allreduce(2, layer(2))

            # ---- final: BN2 apply + leaky (Prelu) + store out ----
            # process TG row-tiles per wide op: fewer instructions and
            # semaphore round-trips in this PE-less serial tail
            TG = 2
            a2w = consts.tile([128, TG * MEM_DIM], F32, name="a2w")
            b2w = consts.tile([128, TG * MEM_DIM], F32, name="b2w")
            for q in range(TG):
                nc.scalar.copy(a2w[:, q * MEM_DIM:(q + 1) * MEM_DIM], a2b)
                nc.scalar.copy(b2w[:, q * MEM_DIM:(q + 1) * MEM_DIM], b2b)
            for i in range(0, nt, TG):
                hw_t = work.tile([128, TG * MEM_DIM], BF16, tag="h2i",
                                 name="h2i", bufs=3)
                for q in range(TG):
                    nc.sync.dma_start(
                        hw_t[:, q * MEM_DIM:(q + 1) * MEM_DIM],
                        h2_dram[(i + q) * 128:(i + q + 1) * 128, :])
                y = work.tile([128, TG * MEM_DIM], F32, tag="y", name="y", bufs=2)
                nc.vector.tensor_mul(y, hw_t, a2w)
                nc.vector.tensor_add(y, y, b2w)
                yo = work.tile([128, TG * MEM_DIM], F32, tag="yo", name="yo",
                               bufs=2)
                nc.scalar.activation(yo, y, AF.Prelu, alpha=LEAKY)
                for q in range(TG):
                    nc.sync.dma_start(
                        out_d[(i + q) * 128:(i + q + 1) * 128, :],
                        yo[:, q * MEM_DIM:(q + 1) * MEM_DIM])

    nc.compile()
    return nc


_CACHE = {}


def _get_nc(n_cores, rows_per_core, use_f32r=True):
    key = (n_cores, rows_per_core, use_f32r)
    if key not in _CACHE:
        _CACHE[key] = build_nc(n_cores, rows_per_core, use_f32r)
    return _CACHE[key]


def kernel(x, mem1, mem2, gamma1, beta1, gamma2, beta2, _trace=False,
           _use_f32r=True, _n_cores=8):
    n_cores = _n_cores
    n, d = x.shape
    rows_per_core = n // n_cores
    nc = _get_nc(n_cores, rows_per_core, _use_f32r)

    in_maps = []
    for c in range(n_cores):
        in_maps.append({
            "x": np.ascontiguousarray(x[c * rows_per_core:(c + 1) * rows_per_core]),
            "mem1": np.ascontiguousarray(mem1),
            "mem2": np.ascontiguousarray(mem2),
            "gamma1": np.ascontiguousarray(gamma1.reshape(1, -1)),
            "beta1": np.ascontiguousarray(beta1.reshape(1, -1)),
            "gamma2": np.ascontiguousarray(gamma2.reshape(1, -1)),
            "beta2": np.ascontiguousarray(beta2.reshape(1, -1)),
        })
    res = run_bass_kernel_spmd(nc, in_maps, list(range(n_cores)), trace=_trace)
    out = np.concatenate([res.results[c]["out"] for c in range(n_cores)], axis=0)
    if _trace:
        return out, res
    return out


# revision 30
# speedup vs baseline: 1.0472x; 1.0472x over previous
"""Trainium2 Bass kernel for nn_CMmodel (retrieval_knn).

Model (per layer, x2):
    sim = cosine(x, mem)                       # [N, 2048]
    S, I = top_k(sim, 10); w = softmax(relu(S))
    h = sum_k w[n,k] * mem[I[n,k]]             # [N, 256]
    h = leaky_relu(batchnorm(h))               # batch stats over ALL N rows

Strategy (8 cores, data-parallel over N):
  - Shard x rows 8 ways; replicate mem banks + BN params.
  - sim via PE matmul, 3-pass f32r exactness scheme (top-10 selection is
    precision-critical: a 10/11 rank swap is a ~50% error on that row, and
    even 1e-6 sim noise swaps ~30 rows of 32k).
  - Exact top-10 threshold t via DVE max8 + match_replace + max8.
  - U = (s>=t)*exp(s-t) via fused DVE scalar_tensor_tensor w/ accum Z.
  - h = U @ mem via PE: U transposed 128x128 on PE (fp32 for layer 1 since
    h1 feeds layer-2 sims; f32r for layer 2), matmul vs raw mem.
  - BatchNorm batch stats via ones-matmul into PSUM, deferred one tile so
    the PE never waits on the ACT/DVE drain chain; AllReduce'd (2KB).
  - ALL activations live in the 'natural_log_exp_and_others' ACT table set
    (Exp, Ln, Prelu, Copy, Square): rsqrt = exp(-0.5*ln), leaky = Prelu.
    Zero table swaps in steady state.
  - mem2 bank prep is emitted after layer 1 so it fills the AllReduce
    bubble; h2 stays SBUF-resident for the final BN2 apply.
"""
import sys

sys.path.insert(0, "/opt/trn_rl_repo")

import numpy as np

import concourse.bacc as bacc
import concourse.mybir as mybir
import concourse.tile as tile
from concourse.bass_utils import run_bass_kernel_spmd
from concourse.masks import make_identity
from concourse.tile import add_dep_helper

F32 = mybir.dt.float32
F32R = mybir.dt.float32r
BF16 = mybir.dt.bfloat16
AF = mybir.ActivationFunctionType
OP = mybir.AluOpType

MEM_DIM = 256
MEM_SIZE = 2048
K_TOP = 10
BN_EPS = 1e-5
LEAKY = 0.01

NJ = MEM_SIZE // 128  # 16 mem-row chunks
NEG_BIG = -1e30


def build_nc(n_cores: int, rows_per_core: int, use_f32r: bool = True):
    """Build the per-core Bass program (SPMD: same program all cores)."""
    nt = rows_per_core // 128  # x tiles per core
    n_total = rows_per_core * n_cores
    MMDT = F32R if use_f32r else F32
    nc = bacc.Bacc("TRN2", target_bir_lowering=False, debug=False,
                   num_devices=n_cores)

    x_d = nc.dram_tensor("x", [rows_per_core, MEM_DIM], F32, kind="ExternalInput")
    mem_d = {
        1: nc.dram_tensor("mem1", [MEM_SIZE, MEM_DIM], F32, kind="ExternalInput"),
        2: nc.dram_tensor("mem2", [MEM_SIZE, MEM_DIM], F32, kind="ExternalInput"),
    }
    gam_d = {
        1: nc.dram_tensor("gamma1", [1, MEM_DIM], F32, kind="ExternalInput"),
        2: nc.dram_tensor("gamma2", [1, MEM_DIM], F32, kind="ExternalInput"),
    }
    bet_d = {
        1: nc.dram_tensor("beta1", [1, MEM_DIM], F32, kind="ExternalInput"),
        2: nc.dram_tensor("beta2", [1, MEM_DIM], F32, kind="ExternalInput"),
    }
    out_d = nc.dram_tensor("out", [rows_per_core, MEM_DIM], F32, kind="ExternalOutput")

    with tile.TileContext(nc) as tc:
        with tc.tile_pool(name="consts", bufs=1) as consts, \
             tc.tile_pool(name="banks", bufs=1) as banks, \
             tc.tile_pool(name="work", bufs=1) as work, \
             tc.tile_pool(name="psum_sim", bufs=2, space="PSUM") as psum_sim, \
             tc.tile_pool(name="psum_tp", bufs=3, space="PSUM") as psum_tp, \
             tc.tile_pool(name="psum_h", bufs=2, space="PSUM") as psum_h_pool, \
             tc.tile_pool(name="psum_st", bufs=1, space="PSUM") as psum_st, \
             tc.tile_pool(name="dram", bufs=1, space="DRAM") as dram:

            # PE emission-order chain: accumulation groups must stay
            # contiguous on PE (interleaved matmuls drop accumulates).
            class _PEChain:
                def __init__(self):
                    self.last = None

                def _chain(self, binst):
                    if self.last is not None:
                        add_dep_helper(binst.ins, self.last.ins, sync=False,
                                       reason="pe-order")
                    self.last = binst
                    return binst

                def matmul(self, *a, **kw):
                    return self._chain(nc.tensor.matmul(*a, **kw))

                def transpose(self, *a, **kw):
                    return self._chain(nc.tensor.transpose(*a, **kw))

            PE = _PEChain()

            # ---------------- constants ----------------
            ident = consts.tile([128, 128], F32)
            make_identity(nc, ident)
            ident_b = consts.tile([128, 128], BF16)
            nc.scalar.copy(ident_b, ident)  # exact: 0/1 values
            ones_col = consts.tile([128, 1], F32)
            nc.vector.memset(ones_col, 1.0)
            ones_col_b = consts.tile([128, 1], BF16)
            nc.vector.memset(ones_col_b, 1.0)
            one_1x1 = consts.tile([1, 1], F32)
            nc.vector.memset(one_1x1, 1.0)
            ones_row = consts.tile([1, 128], F32)
            nc.vector.memset(ones_row, 1.0)
            epsap = consts.tile([1, 1], F32)
            nc.vector.memset(epsap, BN_EPS)

            gb = {}
            for L in (1, 2):
                g = consts.tile([1, MEM_DIM], F32, name=f"gamma_sb{L}")
                b = consts.tile([1, MEM_DIM], F32, name=f"beta_sb{L}")
                nc.sync.dma_start(g, gam_d[L][:])
                nc.sync.dma_start(b, bet_d[L][:])
                gb[L] = (g, b)

            # ---------------- mem banks (prep emitted lazily) ----------------
            # mraw_b[L]: raw mem, natural layout [128, NJ*256] (rhs of h mm)
            # mnT[L,k] : row-normalized mem, transposed, f32r-rounded
            # mnTres   : bf16 residual (m/||m|| - round(m/||m||))
            mraw_b = {}
            mnT = {}
            mnTres = {}
            for L in (1, 2):
                # L1 h-matmul must be fp32-exact (h1 feeds layer-2 sims);
                # L2's only feeds the final output, so bf16 is plenty.
                mraw_b[L] = banks.tile([128, NJ * MEM_DIM],
                                       F32 if L == 1 else BF16, name=f"mraw{L}")
                mnT[L] = [
                    banks.tile([128, MEM_SIZE], MMDT, name=f"mnT{L}_{k}")
                    for k in range(2)
                ]
                mnTres[L] = [
                    banks.tile([128, MEM_SIZE], BF16, name=f"mnTres{L}_{k}")
                    for k in range(2)
                ]

            def prep_bank(L):
                """Load + normalize + transpose one memory bank.

                Batch the norm chain across all 16 chunks (one Ln+Exp rsqrt
                + one batched Newton refine), then per-chunk scale + PE
                transposes. L1 DMAs straight into its fp32 bank; L2's bank
                is bf16, so the fp32 rows are staged (DMA'd twice - once
                for norms, once for scale+convert) through work tiles.
                """
                def stage_in(j, bufs=2):
                    if L == 1:
                        return mraw_b[1][:, j * MEM_DIM:(j + 1) * MEM_DIM]
                    stg = work.tile([128, MEM_DIM], F32, tag="mstg",
                                    name="mstg", bufs=bufs)
                    nc.sync.dma_start(stg, mem_d[L][j * 128:(j + 1) * 128, :])
                    return stg

                if L == 1:
                    for j in range(NJ):
                        nc.sync.dma_start(
                            mraw_b[1][:, j * MEM_DIM:(j + 1) * MEM_DIM],
                            mem_d[1][j * 128:(j + 1) * 128, :])
                mns = work.tile([128, NJ], F32, tag="mns", name="mns", bufs=1)
                for j in range(NJ):
                    src = stage_in(j)
                    msq = work.tile([128, MEM_DIM], F32, tag="sqs", name="sqs",
                                    bufs=2)
                    nc.scalar.activation(msq, src, AF.Square,
                                         accum_out=mns[:, j:j + 1])
                # batched rsqrt, all on DVE (mem-norm precision reorders
                # near-tied sims; 2 Newton steps make it fp32-exact)
                inm = work.tile([128, NJ], F32, tag="inm", name="inm", bufs=1)
                rsqrt_dve(inm, mns, iters=4, seed=1.73)
                for j in range(NJ):
                    src = stage_in(j)
                    if L == 2:  # bf16 bank copy for the h2 matmul rhs
                        nc.vector.tensor_copy(
                            mraw_b[2][:, j * MEM_DIM:(j + 1) * MEM_DIM], src)
                    mnsc = work.tile([128, MEM_DIM], F32, tag="mnsc", name="mnsc",
                                     bufs=2)
                    nc.scalar.mul(mnsc, src, inm[:, j:j + 1])
                    for k in range(2):
                        tp = psum_tp.tile([128, 128], F32, tag="tp2")
                        PE.transpose(tp, mnsc[:, k * 128:(k + 1) * 128], ident)
                        dstT = mnT[L][k][:, j * 128:(j + 1) * 128]
                        nc.scalar.copy(dstT, tp)                   # f32r round
                        # bf16 residual straight from psum - rounded
                        nc.vector.tensor_sub(
                            mnTres[L][k][:, j * 128:(j + 1) * 128],
                            tp, dstT.bitcast(F32))

            # ---------------- persistent stores ----------------
            # h1 and h2 both spill to DRAM (SBUF goes to pipeline buffers)
            h1_dram = nc.dram_tensor("h1buf", [rows_per_core, MEM_DIM], F32)
            h2_dram = nc.dram_tensor("h2buf", [rows_per_core, MEM_DIM], BF16)
            # BN affine params (filled after each AllReduce)
            aT = [consts.tile([128, 1], F32, name=f"aT{k}") for k in range(2)]
            bT = [consts.tile([128, 1], F32, name=f"bT{k}") for k in range(2)]
            a2b = consts.tile([128, MEM_DIM], F32, name="a2b")
            b2b = consts.tile([128, MEM_DIM], F32, name="b2b")

            def rsqrt_dve(dst, src, iters=3, seed="recip", gp=False):
                """dst = 1/sqrt(src), entirely on DVE, keeping Sqrt off the
                ACT engine so the Exp table never gets evicted (each table
                load costs 1.3us). Seed: either a float constant, or
                "recip" = secant fit a*(1/x)+b of sqrt(1/x) over x in
                [60, 400] (max 10% off). Newton steps y*(1.5-0.5*x*y^2)
                converge regardless: 3 iters from 10% -> ~2e-7 rel."""
                shp = list(src.shape)
                eng = nc.gpsimd if gp else nc.vector
                y = work.tile(shp, F32, tag="ny0", name="ny0", bufs=2)
                if seed == "recip":
                    rr = work.tile(shp, F32, tag="nrr", name="nrr", bufs=2)
                    nc.vector.reciprocal(rr, src)
                    eng.tensor_scalar(y, rr, 5.56, 0.0361,
                                      op0=OP.mult, op1=OP.add)
                else:
                    eng.tensor_scalar(y, src, 0.0, float(seed),
                                      op0=OP.mult, op1=OP.add)
                for it in range(iters):
                    t1 = work.tile(shp, F32, tag="nt2", name="nt2", bufs=2)
                    eng.tensor_mul(t1, y, y)
                    eng.tensor_mul(t1, t1, src)
                    eng.tensor_scalar(t1, t1, -0.5, 1.5, op0=OP.mult,
                                      op1=OP.add)
                    out = dst if it == iters - 1 else work.tile(
                        shp, F32, tag=f"ny{it + 1}", name=f"ny{it + 1}", bufs=2)
                    eng.tensor_mul(out, y, t1)
                    y = out

            def stage1a(L, i):
                """lhsT prep for tile i: DMA + transposes + f32r/bf16 splits
                + row-norm rsqrt. Emitted one tile AHEAD of stage1b(i-1)'s
                sims so the ACT queue serves these copies before that tile's
                exp/uts work (otherwise the PE stalls on LDWEIGHTS).
                Small elementwise goes to the idle GpSimd engine."""
                lhsT_f = [
                    work.tile([128, 128], F32, tag=f"lhsTf{k}", name=f"lhsTf{k}", bufs=2)
                    for k in range(2)
                ]
                lhsT_r = [
                    work.tile([128, 128], MMDT, tag=f"lhsTr{k}", name=f"lhsTr{k}", bufs=2)
                    for k in range(2)
                ]
                lhsT_s = [
                    work.tile([128, 128], MMDT, tag=f"lhsTs{k}", name=f"lhsTs{k}", bufs=2)
                    for k in range(2)
                ]
                lhsT_rb = [
                    work.tile([128, 128], BF16, tag=f"lhsTb{k}", name=f"lhsTb{k}", bufs=2)
                    for k in range(2)
                ]
                if L == 1:
                    invn = invn_all[:, i:i + 1]
                    xi = work.tile([128, MEM_DIM], F32, tag="xi", name="xi", bufs=3)
                    nc.sync.dma_start(xi, x_d[i * 128:(i + 1) * 128, :])
                    tpx = psum_tp.tile([128, 256], F32, tag="tp2")
                    for k in range(2):
                        PE.transpose(tpx[:, k * 128:(k + 1) * 128],
                                     xi[:, k * 128:(k + 1) * 128], ident)
                    for k in range(2):
                        nc.scalar.copy(lhsT_f[k], tpx[:, k * 128:(k + 1) * 128])
                else:
                    ns_ps = psum_tp.tile([1, 128], F32, tag="tp2")
                    hsl = work.tile([128, MEM_DIM], F32, tag="h1i", name="h1i", bufs=3)
                    nc.sync.dma_start(hsl, h1_dram[i * 128:(i + 1) * 128, :])
                    sqTs = []
                    tph = psum_tp.tile([128, 256], F32, tag="tp2")
                    for k in range(2):
                        PE.transpose(tph[:, k * 128:(k + 1) * 128],
                                     hsl[:, k * 128:(k + 1) * 128], ident)
                    for k in range(2):
                        # fused BN1 apply + leaky relu at the drain (Prelu
                        # shares the Exp ACT table set; Lrelu doesn't)
                        nc.scalar.activation(
                            lhsT_f[k], tph[:, k * 128:(k + 1) * 128], AF.Prelu,
                            bias=bT[k], scale=aT[k], alpha=LEAKY,
                        )
                        sqT = work.tile([128, 128], F32, tag=f"sqT{k}", name=f"sqT{k}", bufs=2)
                        nc.vector.tensor_mul(sqT, lhsT_f[k], lhsT_f[k])
                        sqTs.append(sqT)
                    for k in range(2):
                        PE.matmul(ns_ps, ones_col, sqTs[k],
                                  start=(k == 0), stop=(k == 1))
                    ns_sb = work.tile([1, 128], F32, tag="ns_sb", name="ns_sb", bufs=2)
                    nc.scalar.copy(ns_sb, ns_ps)
                    nsT = psum_tp.tile([128, 1], F32, tag="tp2")
                    PE.transpose(nsT, ns_sb, one_1x1)
                    invn = work.tile([128, 1], F32, tag="invn", name="invn",
                                     bufs=2)
                    nsS = work.tile([128, 1], F32, tag="nsS", name="nsS", bufs=2)
                    nc.scalar.copy(nsS, nsT)
                    rsqrt_dve(invn, nsS)
                for k in range(2):
                    nc.scalar.copy(lhsT_r[k], lhsT_f[k])   # f32r round
                    nc.vector.tensor_copy(lhsT_rb[k], lhsT_f[k])  # bf16 (pass C)
                    rsd = work.tile([128, 128], F32, tag="rsd", name="rsd", bufs=2)
                    nc.vector.tensor_sub(rsd, lhsT_f[k], lhsT_r[k].bitcast(F32))
                    nc.scalar.copy(lhsT_s[k], rsd)         # f32r residual
                return dict(lhsT_r=lhsT_r, lhsT_s=lhsT_s, lhsT_rb=lhsT_rb,
                            invn=invn)

            def stage1b(L, i, sa):
                """3-pass sim matmul + top-10 threshold + masked exp
                weights for tile i (lhsT prepped by stage1a)."""
                lhsT_r, lhsT_s = sa["lhsT_r"], sa["lhsT_s"]
                lhsT_rb, invn = sa["lhsT_rb"], sa["invn"]
                # 3-pass f32r sim matmul: r(x)@r(m) + res_x@r(m) + x_b@res_m_b
                s_sb = work.tile([128, MEM_SIZE], F32, tag="s_sb", name="s_sb", bufs=2)
                for c in range(4):
                    ps = psum_sim.tile([128, 512], F32, tag="sim")
                    for k in range(2):
                        PE.matmul(ps, lhsT_r[k],
                                  mnT[L][k][:, c * 512:(c + 1) * 512],
                                  start=(k == 0), stop=False)
                    for k in range(2):
                        PE.matmul(ps, lhsT_s[k],
                                  mnT[L][k][:, c * 512:(c + 1) * 512],
                                  start=False, stop=False)
                    for k in range(2):
                        PE.matmul(ps, lhsT_rb[k],
                                  mnTres[L][k][:, c * 512:(c + 1) * 512],
                                  start=False, stop=(k == 1))
                    if c % 2 == 0:
                        nc.vector.tensor_scalar(
                            s_sb[:, c * 512:(c + 1) * 512], ps, invn, None,
                            op0=OP.mult)
                    else:
                        nc.scalar.mul(s_sb[:, c * 512:(c + 1) * 512], ps, invn)

                # exact 10th-largest threshold
                m8a = work.tile([128, 8], F32, tag="m8a", name="m8a", bufs=2)
                nc.vector.max(out=m8a, in_=s_sb)
                s_z = work.tile([128, MEM_SIZE], F32, tag="s_z", name="s_z", bufs=2)
                nc.vector.match_replace(out=s_z, in_to_replace=m8a,
                                        in_values=s_sb, imm_value=NEG_BIG)
                m8b = work.tile([128, 8], F32, tag="m8b", name="m8b", bufs=2)
                nc.vector.max(out=m8b, in_=s_z)
                t_ap = m8b[:, K_TOP - 8 - 1:K_TOP - 8]  # 10th largest
                neg_t = work.tile([128, 1], F32, tag="neg_t", name="neg_t", bufs=2)
                nc.vector.tensor_scalar(neg_t, t_ap, -1.0, None, op0=OP.mult)

                # shifted exp weights, masked, with sum
                e = work.tile([128, MEM_SIZE], F32, tag="e", name="e", bufs=1)
                nc.scalar.activation(e, s_sb, AF.Exp, bias=neg_t, scale=1.0)
                if L == 1:
                    U = s_z  # reuse buffer: s_z is dead after m8b
                else:
                    # bf16 weights: DVE rounds on write, so the PE's bf16
                    # fast path (1 cy/row transposes + h2 matmul) is legal
                    U = work.tile([128, MEM_SIZE], BF16, tag="Ub", name="Ub",
                                  bufs=2)
                Z = work.tile([128, 1], F32, tag="Z", name="Z", bufs=2)
                nc.vector.scalar_tensor_tensor(
                    out=U, in0=s_sb, scalar=t_ap, in1=e,
                    op0=OP.is_ge, op1=OP.mult, accum_out=Z,
                )
                rz = work.tile([128, 1], F32, tag="rz", name="rz", bufs=2)
                nc.vector.reciprocal(rz, Z)
                return dict(U=U, rz=rz)

            def stage2(L, i, st, stats_acc, pend):
                """U transposes + h = (U/Z) @ mem + BN batch-stat partials.

                The stats matmuls for THIS tile are deferred: they're pushed
                on `pend` and emitted by the NEXT stage2 call (or the flush),
                so the PE never waits on the dst/sqh drain chain.
                """
                U, rz = st["U"], st["rz"]
                # layer 1 feeds layer-2 sims: must stay fp32-exact.
                # layer 2 only feeds the final output: bf16 is plenty and
                # transposes at 1 cy/row instead of fp32's 2.5.
                ut_dt = F32 if L == 1 else BF16
                tp_ident = ident if L == 1 else ident_b
                # paired transposes -> one [128,256] drain per pair
                uts = []
                for c2 in range(NJ // 2):
                    tp2 = psum_tp.tile([128, 256], ut_dt, tag="tp2")
                    PE.transpose(tp2[:, 0:128],
                                 U[:, (2 * c2) * 128:(2 * c2 + 1) * 128], tp_ident)
                    PE.transpose(tp2[:, 128:256],
                                 U[:, (2 * c2 + 1) * 128:(2 * c2 + 2) * 128], tp_ident)
                    utp = work.tile([128, 256], ut_dt, tag="ut", name="ut",
                                    bufs=NJ // 2 + 2)
                    nc.scalar.copy(utp, tp2)
                    uts.append(utp)
                hp = psum_h_pool.tile([128, MEM_DIM], F32, tag="hp")
                for c in range(NJ):
                    PE.matmul(
                        hp, uts[c // 2][:, (c % 2) * 128:(c % 2 + 1) * 128],
                        mraw_b[L][:, c * MEM_DIM:(c + 1) * MEM_DIM],
                        start=(c == 0), stop=(c == NJ - 1),
                    )
                # drain h (normalized by Z) + square; stats matmuls deferred.
                # h2 is drained + spilled + stat'd in bf16 (halves the tail
                # DMA; BN2 normalizes the rounded values consistently).
                dst = work.tile([128, MEM_DIM], F32 if L == 1 else BF16,
                                tag="h2o", name="h2o", bufs=4)
                nc.scalar.mul(dst, hp, rz)
                h_dram = h1_dram if L == 1 else h2_dram
                nc.sync.dma_start(h_dram[i * 128:(i + 1) * 128, :], dst)
                sqh = work.tile([128, MEM_DIM], F32, tag="sqh", name="sqh", bufs=3)
                nc.vector.tensor_mul(sqh, dst, dst)
                emit_pending_stats(L, stats_acc, pend)
                pend.append((dst, sqh))

            def emit_pending_stats(L, stats_acc, pend):
                ones_l = ones_col if L == 1 else ones_col_b
                while pend:
                    dst, sqh = pend.pop(0)
                    pd = psum_st.tile([1, 512], F32, tag="st")
                    PE.matmul(pd[:, 0:MEM_DIM], ones_l, dst,
                              start=True, stop=True)
                    PE.matmul(pd[:, MEM_DIM:2 * MEM_DIM], ones_col, sqh,
                              start=True, stop=True)
                    nc.vector.tensor_add(stats_acc, stats_acc, pd)

            def layer(L):
                stats_acc = work.tile([1, 512], F32, tag=f"stacc{L}", bufs=1,
                                      name=f"stats_acc{L}")
                nc.vector.memset(stats_acc, 0.0)
                pend = []
                prev = None
                sa = stage1a(L, 0)
                for i in range(nt):
                    sa_next = stage1a(L, i + 1) if i + 1 < nt else None
                    st = stage1b(L, i, sa)
                    sa = sa_next
                    if prev is not None:
                        stage2(L, i - 1, prev, stats_acc, pend)
                    prev = st
                stage2(L, nt - 1, prev, stats_acc, pend)
                emit_pending_stats(L, stats_acc, pend)
                return stats_acc

            def bn_allreduce(L, stats_acc):
                gamma_sb, beta_sb = gb[L]
                stats_sb = stats_acc
                ar_in = dram.tile([1, 512], F32, name=f"ar_in{L}")
                ar_out = dram.tile([1, 512], F32, addr_space="Shared",
                                   name=f"ar_out{L}")
                nc.sync.dma_start(ar_in, stats_sb)
                nc.gpsimd.collective_compute(
                    "AllReduce", OP.add,
                    replica_groups=[list(range(n_cores))],
                    ins=[ar_in[:]], outs=[ar_out[:]],
                )
                gst = work.tile([1, 512], F32, tag="gst", name="gst", bufs=1)
                nc.sync.dma_start(gst, ar_out)

                ab = work.tile([1, 512], F32, tag="ab", name="ab", bufs=1)
                a_ap, b_ap = ab[:, 0:MEM_DIM], ab[:, MEM_DIM:512]
                mu = work.tile([1, MEM_DIM], F32, tag="mu", name="mu", bufs=1)
                nc.vector.tensor_scalar(mu, gst[:, 0:MEM_DIM], 1.0 / n_total,
                                        None, op0=OP.mult)
                ex2 = work.tile([1, MEM_DIM], F32, tag="ex2", name="ex2", bufs=1)
                nc.vector.tensor_scalar(ex2, gst[:, MEM_DIM:512], 1.0 / n_total,
                                        None, op0=OP.mult)
                musq = work.tile([1, MEM_DIM], F32, tag="musq", name="musq", bufs=1)
                nc.vector.tensor_mul(musq, mu, mu)
                var = work.tile([1, MEM_DIM], F32, tag="var", name="var", bufs=1)
                nc.vector.tensor_sub(var, ex2, musq)
                sd = work.tile([1, MEM_DIM], F32, tag="sd", name="sd", bufs=1)
                nc.scalar.activation(sd, var, AF.Sqrt, bias=epsap)
                isd = work.tile([1, MEM_DIM], F32, tag="isd", name="isd", bufs=1)
                nc.vector.reciprocal(isd, sd)
                nc.vector.tensor_mul(a_ap, gamma_sb, isd)
                mua = work.tile([1, MEM_DIM], F32, tag="mua", name="mua", bufs=1)
                nc.vector.tensor_mul(mua, mu, a_ap)
                nc.vector.tensor_sub(b_ap, beta_sb, mua)

                if L == 1:
                    # per-partition (transposed-layout) affine params
                    for k in range(2):
                        for src, dstp in ((a_ap, aT[k]), (b_ap, bT[k])):
                            tp = psum_tp.tile([128, 1], F32, tag="tp2")
                            PE.transpose(
                                tp, src[:, k * 128:(k + 1) * 128], one_1x1)
                            nc.scalar.copy(dstp, tp)
                else:
                    # broadcast across partitions (row-layout affine)
                    bc = psum_sim.tile([128, 512], F32, tag="sim")
                    PE.matmul(bc, ones_row, ab, start=True, stop=True)
                    nc.scalar.copy(a2b, bc[:, 0:MEM_DIM])
                    nc.scalar.copy(b2b, bc[:, MEM_DIM:512])

            # L1 row-norm prologue: second read of x during the prep
            # window computes every tile's 1/||x|| in one batched chain,
            # so the steady-state DVE queue holds only topk+weights work.
            invn_all = consts.tile([128, nt], F32, name="invn_all")
            xns_all = consts.tile([128, nt], F32, name="xns_all")
            prep_bank(1)
            for i in range(nt):
                xpr = work.tile([128, MEM_DIM], F32, tag="xpr", name="xpr",
                                bufs=3)
                nc.sync.dma_start(xpr, x_d[i * 128:(i + 1) * 128, :])
                xsq = work.tile([128, MEM_DIM], F32, tag="sqs", name="sqs",
                                bufs=2)
                nc.vector.scalar_tensor_tensor(
                    out=xsq, in0=xpr, scalar=0.0, in1=xpr,
                    op0=OP.add, op1=OP.mult, accum_out=xns_all[:, i:i + 1])
            rsqrt_dve(invn_all, xns_all, iters=3, seed="recip")
            s1 = layer(1)
            prep_bank(2)  # fills the AllReduce bubble + L1 tail
            bn_allreduce(1, s1)
            bn_allreduce(2, layer(2))

            # ---- final: BN2 apply + leaky (Prelu) + store out ----
            # process TG row-tiles per wide op: fewer instructions and
            # semaphore round-trips in this PE-less serial tail
            TG = 2
            a2w = consts.tile([128, TG * MEM_DIM], F32, name="a2w")
            b2w = consts.tile([128, TG * MEM_DIM], F32, name="b2w")
            for q in range(TG):
                nc.scalar.copy(a2w[:, q * MEM_DIM:(q + 1) * MEM_DIM], a2b)
                nc.scalar.copy(b2w[:, q * MEM_DIM:(q + 1) * MEM_DIM], b2b)
            for i in range(0, nt, TG):
                hw_t = work.tile([128, TG * MEM_DIM], BF16, tag="h2i",
                                 name="h2i", bufs=3)
                for q in range(TG):
                    nc.sync.dma_start(
                        hw_t[:, q * MEM_DIM:(q + 1) * MEM_DIM],
                        h2_dram[(i + q) * 128:(i + q + 1) * 128, :])
                y = work.tile([128, TG * MEM_DIM], F32, tag="y", name="y", bufs=2)
                nc.vector.tensor_mul(y, hw_t, a2w)
                nc.vector.tensor_add(y, y, b2w)
                yo = work.tile([128, TG * MEM_DIM], F32, tag="yo", name="yo",
                               bufs=2)
                nc.scalar.activation(yo, y, AF.Prelu, alpha=LEAKY)
                for q in range(TG):
                    nc.sync.dma_start(
                        out_d[(i + q) * 128:(i + q + 1) * 128, :],
                        yo[:, q * MEM_DIM:(q + 1) * MEM_DIM])

    nc.compile()
    return nc


_CACHE = {}


def _get_nc(n_cores, rows_per_core, use_f32r=True):
    key = (n_cores, rows_per_core, use_f32r)
    if key not in _CACHE:
        _CACHE[key] = build_nc(n_cores, rows_per_core, use_f32r)
    return _CACHE[key]


def kernel(x, mem1, mem2, gamma1, beta1, gamma2, beta2, _trace=False,
           _use_f32r=True, _n_cores=8):
    n_cores = _n_cores
    n, d = x.shape
    rows_per_core = n // n_cores
    nc = _get_nc(n_cores, rows_per_core, _use_f32r)

    in_maps = []
    for c in range(n_cores):
        in_maps.append({
            "x": np.ascontiguousarray(x[c * rows_per_core:(c + 1) * rows_per_core]),
            "mem1": np.ascontiguousarray(mem1),
            "mem2": np.ascontiguousarray(mem2),
            "gamma1": np.ascontiguousarray(gamma1.reshape(1, -1)),
            "beta1": np.ascontiguousarray(beta1.reshape(1, -1)),
            "gamma2": np.ascontiguousarray(gamma2.reshape(1, -1)),
            "beta2": np.ascontiguousarray(beta2.reshape(1, -1)),
        })
    res = run_bass_kernel_spmd(nc, in_maps, list(range(n_cores)), trace=_trace)
    out = np.concatenate([res.results[c]["out"] for c in range(n_cores)], axis=0)
    if _trace:
        return out, res
    return out


# revision 31
# speedup vs baseline: 1.0494x; 1.0021x over previous
"""Trainium2 Bass kernel for nn_CMmodel (retrieval_knn).

Model (per layer, x2):
    sim = cosine(x, mem)                       # [N, 2048]
    S, I = top_k(sim, 10); w = softmax(relu(S))
    h = sum_k w[n,k] * mem[I[n,k]]             # [N, 256]
    h = leaky_relu(batchnorm(h))               # batch stats over ALL N rows

Strategy (8 cores, data-parallel over N):
  - Shard x rows 8 ways; replicate mem banks + BN params.
  - sim via PE matmul, 3-pass f32r exactness scheme (top-10 selection is
    precision-critical: a 10/11 rank swap is a ~50% error on that row, and
    even 1e-6 sim noise swaps ~30 rows of 32k).
  - Exact top-10 threshold t via DVE max8 + match_replace + max8.
  - U = (s>=t)*exp(s-t) via fused DVE scalar_tensor_tensor w/ accum Z.
  - h = U @ mem via PE: U transposed 128x128 on PE (fp32 for layer 1 since
    h1 feeds layer-2 sims; f32r for layer 2), matmul vs raw mem.
  - BatchNorm batch stats via ones-matmul into PSUM, deferred one tile so
    the PE never waits on the ACT/DVE drain chain; AllReduce'd (2KB).
  - ALL activations live in the 'natural_log_exp_and_others' ACT table set
    (Exp, Ln, Prelu, Copy, Square): rsqrt = exp(-0.5*ln), leaky = Prelu.
    Zero table swaps in steady state.
  - mem2 bank prep is emitted after layer 1 so it fills the AllReduce
    bubble; h2 stays SBUF-resident for the final BN2 apply.
"""
import sys

sys.path.insert(0, "/opt/trn_rl_repo")

import numpy as np

import concourse.bacc as bacc
import concourse.mybir as mybir
import concourse.tile as tile
from concourse.bass_utils import run_bass_kernel_spmd
from concourse.masks import make_identity
from concourse.tile import add_dep_helper

F32 = mybir.dt.float32
F32R = mybir.dt.float32r
BF16 = mybir.dt.bfloat16
AF = mybir.ActivationFunctionType
OP = mybir.AluOpType

MEM_DIM = 256
MEM_SIZE = 2048
K_TOP = 10
BN_EPS = 1e-5
LEAKY = 0.01

NJ = MEM_SIZE // 128  # 16 mem-row chunks
NEG_BIG = -1e30


def build_nc(n_cores: int, rows_per_core: int, use_f32r: bool = True):
    """Build the per-core Bass program (SPMD: same program all cores)."""
    nt = rows_per_core // 128  # x tiles per core
    n_total = rows_per_core * n_cores
    MMDT = F32R if use_f32r else F32
    nc = bacc.Bacc("TRN2", target_bir_lowering=False, debug=False,
                   num_devices=n_cores)

    x_d = nc.dram_tensor("x", [rows_per_core, MEM_DIM], F32, kind="ExternalInput")
    mem_d = {
        1: nc.dram_tensor("mem1", [MEM_SIZE, MEM_DIM], F32, kind="ExternalInput"),
        2: nc.dram_tensor("mem2", [MEM_SIZE, MEM_DIM], F32, kind="ExternalInput"),
    }
    gam_d = {
        1: nc.dram_tensor("gamma1", [1, MEM_DIM], F32, kind="ExternalInput"),
        2: nc.dram_tensor("gamma2", [1, MEM_DIM], F32, kind="ExternalInput"),
    }
    bet_d = {
        1: nc.dram_tensor("beta1", [1, MEM_DIM], F32, kind="ExternalInput"),
        2: nc.dram_tensor("beta2", [1, MEM_DIM], F32, kind="ExternalInput"),
    }
    out_d = nc.dram_tensor("out", [rows_per_core, MEM_DIM], F32, kind="ExternalOutput")

    with tile.TileContext(nc) as tc:
        with tc.tile_pool(name="consts", bufs=1) as consts, \
             tc.tile_pool(name="banks", bufs=1) as banks, \
             tc.tile_pool(name="work", bufs=1) as work, \
             tc.tile_pool(name="psum_sim", bufs=2, space="PSUM") as psum_sim, \
             tc.tile_pool(name="psum_tp", bufs=3, space="PSUM") as psum_tp, \
             tc.tile_pool(name="psum_h", bufs=2, space="PSUM") as psum_h_pool, \
             tc.tile_pool(name="psum_st", bufs=1, space="PSUM") as psum_st, \
             tc.tile_pool(name="dram", bufs=1, space="DRAM") as dram:

            # PE emission-order chain: accumulation groups must stay
            # contiguous on PE (interleaved matmuls drop accumulates).
            class _PEChain:
                def __init__(self):
                    self.last = None

                def _chain(self, binst):
                    if self.last is not None:
                        add_dep_helper(binst.ins, self.last.ins, sync=False,
                                       reason="pe-order")
                    self.last = binst
                    return binst

                def matmul(self, *a, **kw):
                    return self._chain(nc.tensor.matmul(*a, **kw))

                def transpose(self, *a, **kw):
                    return self._chain(nc.tensor.transpose(*a, **kw))

            PE = _PEChain()

            # ---------------- constants ----------------
            ident = consts.tile([128, 128], F32)
            make_identity(nc, ident)
            ident_b = consts.tile([128, 128], BF16)
            nc.scalar.copy(ident_b, ident)  # exact: 0/1 values
            ones_col = consts.tile([128, 1], F32)
            nc.vector.memset(ones_col, 1.0)
            ones_col_b = consts.tile([128, 1], BF16)
            nc.vector.memset(ones_col_b, 1.0)
            one_1x1 = consts.tile([1, 1], F32)
            nc.vector.memset(one_1x1, 1.0)
            ones_row = consts.tile([1, 128], F32)
            nc.vector.memset(ones_row, 1.0)
            epsap = consts.tile([1, 1], F32)
            nc.vector.memset(epsap, BN_EPS)

            gb = {}
            for L in (1, 2):
                g = consts.tile([1, MEM_DIM], F32, name=f"gamma_sb{L}")
                b = consts.tile([1, MEM_DIM], F32, name=f"beta_sb{L}")
                nc.sync.dma_start(g, gam_d[L][:])
                nc.sync.dma_start(b, bet_d[L][:])
                gb[L] = (g, b)

            # ---------------- mem banks (prep emitted lazily) ----------------
            # mraw_b[L]: raw mem, natural layout [128, NJ*256] (rhs of h mm)
            # mnT[L,k] : row-normalized mem, transposed, f32r-rounded
            # mnTres   : bf16 residual (m/||m|| - round(m/||m||))
            mraw_b = {}
            mnT = {}
            mnTres = {}
            for L in (1, 2):
                # L1 h-matmul must be fp32-exact (h1 feeds layer-2 sims);
                # L2's only feeds the final output, so bf16 is plenty.
                mraw_b[L] = banks.tile([128, NJ * MEM_DIM],
                                       F32 if L == 1 else BF16, name=f"mraw{L}")
                mnT[L] = [
                    banks.tile([128, MEM_SIZE], MMDT, name=f"mnT{L}_{k}")
                    for k in range(2)
                ]
                mnTres[L] = [
                    banks.tile([128, MEM_SIZE], BF16, name=f"mnTres{L}_{k}")
                    for k in range(2)
                ]

            def prep_bank(L):
                """Load + normalize + transpose one memory bank.

                Batch the norm chain across all 16 chunks (one Ln+Exp rsqrt
                + one batched Newton refine), then per-chunk scale + PE
                transposes. L1 DMAs straight into its fp32 bank; L2's bank
                is bf16, so the fp32 rows are staged (DMA'd twice - once
                for norms, once for scale+convert) through work tiles.
                """
                def stage_in(j, bufs=2):
                    if L == 1:
                        return mraw_b[1][:, j * MEM_DIM:(j + 1) * MEM_DIM]
                    stg = work.tile([128, MEM_DIM], F32, tag="mstg",
                                    name="mstg", bufs=bufs)
                    nc.sync.dma_start(stg, mem_d[L][j * 128:(j + 1) * 128, :])
                    return stg

                if L == 1:
                    for j in range(NJ):
                        nc.sync.dma_start(
                            mraw_b[1][:, j * MEM_DIM:(j + 1) * MEM_DIM],
                            mem_d[1][j * 128:(j + 1) * 128, :])
                mns = work.tile([128, NJ], F32, tag="mns", name="mns", bufs=1)
                for j in range(NJ):
                    src = stage_in(j)
                    msq = work.tile([128, MEM_DIM], F32, tag="sqs", name="sqs",
                                    bufs=2)
                    nc.scalar.activation(msq, src, AF.Square,
                                         accum_out=mns[:, j:j + 1])
                # batched rsqrt, all on DVE (mem-norm precision reorders
                # near-tied sims; 2 Newton steps make it fp32-exact)
                inm = work.tile([128, NJ], F32, tag="inm", name="inm", bufs=1)
                rsqrt_dve(inm, mns, iters=4, seed=1.73)
                for j in range(NJ):
                    src = stage_in(j)
                    if L == 2:  # bf16 bank copy for the h2 matmul rhs
                        nc.vector.tensor_copy(
                            mraw_b[2][:, j * MEM_DIM:(j + 1) * MEM_DIM], src)
                    mnsc = work.tile([128, MEM_DIM], F32, tag="mnsc", name="mnsc",
                                     bufs=2)
                    nc.scalar.mul(mnsc, src, inm[:, j:j + 1])
                    for k in range(2):
                        tp = psum_tp.tile([128, 128], F32, tag="tp2")
                        PE.transpose(tp, mnsc[:, k * 128:(k + 1) * 128], ident)
                        dstT = mnT[L][k][:, j * 128:(j + 1) * 128]
                        nc.scalar.copy(dstT, tp)                   # f32r round
                        # bf16 residual straight from psum - rounded
                        nc.vector.tensor_sub(
                            mnTres[L][k][:, j * 128:(j + 1) * 128],
                            tp, dstT.bitcast(F32))

            # ---------------- persistent stores ----------------
            # h1 and h2 both spill to DRAM (SBUF goes to pipeline buffers)
            h1_dram = nc.dram_tensor("h1buf", [rows_per_core, MEM_DIM], F32)
            h2_dram = nc.dram_tensor("h2buf", [rows_per_core, MEM_DIM], BF16)
            # BN affine params (filled after each AllReduce)
            aT = [consts.tile([128, 1], F32, name=f"aT{k}") for k in range(2)]
            bT = [consts.tile([128, 1], F32, name=f"bT{k}") for k in range(2)]
            a2b = consts.tile([128, MEM_DIM], F32, name="a2b")
            b2b = consts.tile([128, MEM_DIM], F32, name="b2b")

            def rsqrt_dve(dst, src, iters=3, seed="recip", gp=False):
                """dst = 1/sqrt(src), entirely on DVE, keeping Sqrt off the
                ACT engine so the Exp table never gets evicted (each table
                load costs 1.3us). Seed: either a float constant, or
                "recip" = secant fit a*(1/x)+b of sqrt(1/x) over x in
                [60, 400] (max 10% off). Newton steps y*(1.5-0.5*x*y^2)
                converge regardless: 3 iters from 10% -> ~2e-7 rel."""
                shp = list(src.shape)
                eng = nc.gpsimd if gp else nc.vector
                y = work.tile(shp, F32, tag="ny0", name="ny0", bufs=2)
                if seed == "recip":
                    rr = work.tile(shp, F32, tag="nrr", name="nrr", bufs=2)
                    nc.vector.reciprocal(rr, src)
                    eng.tensor_scalar(y, rr, 5.56, 0.0361,
                                      op0=OP.mult, op1=OP.add)
                else:
                    eng.tensor_scalar(y, src, 0.0, float(seed),
                                      op0=OP.mult, op1=OP.add)
                for it in range(iters):
                    t1 = work.tile(shp, F32, tag="nt2", name="nt2", bufs=2)
                    eng.tensor_mul(t1, y, y)
                    eng.tensor_mul(t1, t1, src)
                    eng.tensor_scalar(t1, t1, -0.5, 1.5, op0=OP.mult,
                                      op1=OP.add)
                    out = dst if it == iters - 1 else work.tile(
                        shp, F32, tag=f"ny{it + 1}", name=f"ny{it + 1}", bufs=2)
                    eng.tensor_mul(out, y, t1)
                    y = out

            def stage1a(L, i):
                """lhsT prep for tile i: DMA + transposes + f32r/bf16 splits
                + row-norm rsqrt. Emitted one tile AHEAD of stage1b(i-1)'s
                sims so the ACT queue serves these copies before that tile's
                exp/uts work (otherwise the PE stalls on LDWEIGHTS).
                Small elementwise goes to the idle GpSimd engine."""
                lhsT_f = [
                    work.tile([128, 128], F32, tag=f"lhsTf{k}", name=f"lhsTf{k}", bufs=2)
                    for k in range(2)
                ]
                lhsT_r = [
                    work.tile([128, 128], MMDT, tag=f"lhsTr{k}", name=f"lhsTr{k}", bufs=2)
                    for k in range(2)
                ]
                lhsT_s = [
                    work.tile([128, 128], MMDT, tag=f"lhsTs{k}", name=f"lhsTs{k}", bufs=2)
                    for k in range(2)
                ]
                lhsT_rb = [
                    work.tile([128, 128], BF16, tag=f"lhsTb{k}", name=f"lhsTb{k}", bufs=2)
                    for k in range(2)
                ]
                if L == 1:
                    invn = invn_all[:, i:i + 1]
                    xi = work.tile([128, MEM_DIM], F32, tag="xi", name="xi", bufs=3)
                    nc.sync.dma_start(xi, x_d[i * 128:(i + 1) * 128, :])
                    tpx = psum_tp.tile([128, 256], F32, tag="tp2")
                    for k in range(2):
                        PE.transpose(tpx[:, k * 128:(k + 1) * 128],
                                     xi[:, k * 128:(k + 1) * 128], ident)
                    for k in range(2):
                        nc.scalar.copy(lhsT_f[k], tpx[:, k * 128:(k + 1) * 128])
                else:
                    ns_ps = psum_tp.tile([1, 128], F32, tag="tp2")
                    hsl = work.tile([128, MEM_DIM], F32, tag="h1i", name="h1i", bufs=3)
                    nc.sync.dma_start(hsl, h1_dram[i * 128:(i + 1) * 128, :])
                    sqTs = []
                    tph = psum_tp.tile([128, 256], F32, tag="tp2")
                    for k in range(2):
                        PE.transpose(tph[:, k * 128:(k + 1) * 128],
                                     hsl[:, k * 128:(k + 1) * 128], ident)
                    for k in range(2):
                        # fused BN1 apply + leaky relu at the drain (Prelu
                        # shares the Exp ACT table set; Lrelu doesn't)
                        nc.scalar.activation(
                            lhsT_f[k], tph[:, k * 128:(k + 1) * 128], AF.Prelu,
                            bias=bT[k], scale=aT[k], alpha=LEAKY,
                        )
                        sqT = work.tile([128, 128], F32, tag=f"sqT{k}", name=f"sqT{k}", bufs=2)
                        nc.vector.tensor_mul(sqT, lhsT_f[k], lhsT_f[k])
                        sqTs.append(sqT)
                    for k in range(2):
                        PE.matmul(ns_ps, ones_col, sqTs[k],
                                  start=(k == 0), stop=(k == 1))
                    ns_sb = work.tile([1, 128], F32, tag="ns_sb", name="ns_sb", bufs=2)
                    nc.scalar.copy(ns_sb, ns_ps)
                    nsT = psum_tp.tile([128, 1], F32, tag="tp2")
                    PE.transpose(nsT, ns_sb, one_1x1)
                    invn = work.tile([128, 1], F32, tag="invn", name="invn",
                                     bufs=2)
                    nsS = work.tile([128, 1], F32, tag="nsS", name="nsS", bufs=2)
                    nc.scalar.copy(nsS, nsT)
                    rsqrt_dve(invn, nsS)
                for k in range(2):
                    nc.scalar.copy(lhsT_r[k], lhsT_f[k])   # f32r round
                    nc.vector.tensor_copy(lhsT_rb[k], lhsT_f[k])  # bf16 (pass C)
                    rsd = work.tile([128, 128], F32, tag="rsd", name="rsd", bufs=2)
                    nc.vector.tensor_sub(rsd, lhsT_f[k], lhsT_r[k].bitcast(F32))
                    nc.scalar.copy(lhsT_s[k], rsd)         # f32r residual
                return dict(lhsT_r=lhsT_r, lhsT_s=lhsT_s, lhsT_rb=lhsT_rb,
                            invn=invn)

            def stage1b(L, i, sa):
                """3-pass sim matmul + top-10 threshold + masked exp
                weights for tile i (lhsT prepped by stage1a)."""
                lhsT_r, lhsT_s = sa["lhsT_r"], sa["lhsT_s"]
                lhsT_rb, invn = sa["lhsT_rb"], sa["invn"]
                # 3-pass f32r sim matmul: r(x)@r(m) + res_x@r(m) + x_b@res_m_b
                s_sb = work.tile([128, MEM_SIZE], F32, tag="s_sb", name="s_sb", bufs=2)
                for c in range(4):
                    ps = psum_sim.tile([128, 512], F32, tag="sim")
                    for k in range(2):
                        PE.matmul(ps, lhsT_r[k],
                                  mnT[L][k][:, c * 512:(c + 1) * 512],
                                  start=(k == 0), stop=False)
                    for k in range(2):
                        PE.matmul(ps, lhsT_s[k],
                                  mnT[L][k][:, c * 512:(c + 1) * 512],
                                  start=False, stop=False)
                    for k in range(2):
                        PE.matmul(ps, lhsT_rb[k],
                                  mnTres[L][k][:, c * 512:(c + 1) * 512],
                                  start=False, stop=(k == 1))
                    nc.scalar.mul(s_sb[:, c * 512:(c + 1) * 512], ps, invn)

                # exact 10th-largest threshold
                m8a = work.tile([128, 8], F32, tag="m8a", name="m8a", bufs=2)
                nc.vector.max(out=m8a, in_=s_sb)
                s_z = work.tile([128, MEM_SIZE], F32, tag="s_z", name="s_z", bufs=2)
                nc.vector.match_replace(out=s_z, in_to_replace=m8a,
                                        in_values=s_sb, imm_value=NEG_BIG)
                m8b = work.tile([128, 8], F32, tag="m8b", name="m8b", bufs=2)
                nc.vector.max(out=m8b, in_=s_z)
                t_ap = m8b[:, K_TOP - 8 - 1:K_TOP - 8]  # 10th largest
                neg_t = work.tile([128, 1], F32, tag="neg_t", name="neg_t", bufs=2)
                nc.vector.tensor_scalar(neg_t, t_ap, -1.0, None, op0=OP.mult)

                # shifted exp weights, masked, with sum
                e = work.tile([128, MEM_SIZE], F32, tag="e", name="e", bufs=1)
                nc.scalar.activation(e, s_sb, AF.Exp, bias=neg_t, scale=1.0)
                if L == 1:
                    U = s_z  # reuse buffer: s_z is dead after m8b
                else:
                    # bf16 weights: DVE rounds on write, so the PE's bf16
                    # fast path (1 cy/row transposes + h2 matmul) is legal
                    U = work.tile([128, MEM_SIZE], BF16, tag="Ub", name="Ub",
                                  bufs=2)
                Z = work.tile([128, 1], F32, tag="Z", name="Z", bufs=2)
                nc.vector.scalar_tensor_tensor(
                    out=U, in0=s_sb, scalar=t_ap, in1=e,
                    op0=OP.is_ge, op1=OP.mult, accum_out=Z,
                )
                rz = work.tile([128, 1], F32, tag="rz", name="rz", bufs=2)
                nc.vector.reciprocal(rz, Z)
                return dict(U=U, rz=rz)

            def stage2(L, i, st, stats_acc, pend):
                """U transposes + h = (U/Z) @ mem + BN batch-stat partials.

                The stats matmuls for THIS tile are deferred: they're pushed
                on `pend` and emitted by the NEXT stage2 call (or the flush),
                so the PE never waits on the dst/sqh drain chain.
                """
                U, rz = st["U"], st["rz"]
                # layer 1 feeds layer-2 sims: must stay fp32-exact.
                # layer 2 only feeds the final output: bf16 is plenty and
                # transposes at 1 cy/row instead of fp32's 2.5.
                ut_dt = F32 if L == 1 else BF16
                tp_ident = ident if L == 1 else ident_b
                # paired transposes -> one [128,256] drain per pair
                uts = []
                for c2 in range(NJ // 2):
                    tp2 = psum_tp.tile([128, 256], ut_dt, tag="tp2")
                    PE.transpose(tp2[:, 0:128],
                                 U[:, (2 * c2) * 128:(2 * c2 + 1) * 128], tp_ident)
                    PE.transpose(tp2[:, 128:256],
                                 U[:, (2 * c2 + 1) * 128:(2 * c2 + 2) * 128], tp_ident)
                    utp = work.tile([128, 256], ut_dt, tag="ut", name="ut",
                                    bufs=NJ // 2 + 2)
                    nc.scalar.copy(utp, tp2)
                    uts.append(utp)
                hp = psum_h_pool.tile([128, MEM_DIM], F32, tag="hp")
                for c in range(NJ):
                    PE.matmul(
                        hp, uts[c // 2][:, (c % 2) * 128:(c % 2 + 1) * 128],
                        mraw_b[L][:, c * MEM_DIM:(c + 1) * MEM_DIM],
                        start=(c == 0), stop=(c == NJ - 1),
                    )
                # drain h (normalized by Z) + square; stats matmuls deferred.
                # h2 is drained + spilled + stat'd in bf16 (halves the tail
                # DMA; BN2 normalizes the rounded values consistently).
                dst = work.tile([128, MEM_DIM], F32 if L == 1 else BF16,
                                tag="h2o", name="h2o", bufs=4)
                nc.scalar.mul(dst, hp, rz)
                h_dram = h1_dram if L == 1 else h2_dram
                nc.sync.dma_start(h_dram[i * 128:(i + 1) * 128, :], dst)
                sqh = work.tile([128, MEM_DIM], F32, tag="sqh", name="sqh", bufs=3)
                nc.vector.tensor_mul(sqh, dst, dst)
                emit_pending_stats(L, stats_acc, pend)
                pend.append((dst, sqh))

            def emit_pending_stats(L, stats_acc, pend):
                ones_l = ones_col if L == 1 else ones_col_b
                while pend:
                    dst, sqh = pend.pop(0)
                    pd = psum_st.tile([1, 512], F32, tag="st")
                    PE.matmul(pd[:, 0:MEM_DIM], ones_l, dst,
                              start=True, stop=True)
                    PE.matmul(pd[:, MEM_DIM:2 * MEM_DIM], ones_col, sqh,
                              start=True, stop=True)
                    nc.vector.tensor_add(stats_acc, stats_acc, pd)

            def layer(L):
                stats_acc = work.tile([1, 512], F32, tag=f"stacc{L}", bufs=1,
                                      name=f"stats_acc{L}")
                nc.vector.memset(stats_acc, 0.0)
                pend = []
                prev = None
                sa = stage1a(L, 0)
                for i in range(nt):
                    sa_next = stage1a(L, i + 1) if i + 1 < nt else None
                    st = stage1b(L, i, sa)
                    sa = sa_next
                    if prev is not None:
                        stage2(L, i - 1, prev, stats_acc, pend)
                    prev = st
                stage2(L, nt - 1, prev, stats_acc, pend)
                emit_pending_stats(L, stats_acc, pend)
                return stats_acc

            def bn_allreduce(L, stats_acc):
                gamma_sb, beta_sb = gb[L]
                stats_sb = stats_acc
                ar_in = dram.tile([1, 512], F32, name=f"ar_in{L}")
                ar_out = dram.tile([1, 512], F32, addr_space="Shared",
                                   name=f"ar_out{L}")
                nc.sync.dma_start(ar_in, stats_sb)
                nc.gpsimd.collective_compute(
                    "AllReduce", OP.add,
                    replica_groups=[list(range(n_cores))],
                    ins=[ar_in[:]], outs=[ar_out[:]],
                )
                gst = work.tile([1, 512], F32, tag="gst", name="gst", bufs=1)
                nc.sync.dma_start(gst, ar_out)

                ab = work.tile([1, 512], F32, tag="ab", name="ab", bufs=1)
                a_ap, b_ap = ab[:, 0:MEM_DIM], ab[:, MEM_DIM:512]
                mu = work.tile([1, MEM_DIM], F32, tag="mu", name="mu", bufs=1)
                nc.vector.tensor_scalar(mu, gst[:, 0:MEM_DIM], 1.0 / n_total,
                                        None, op0=OP.mult)
                ex2 = work.tile([1, MEM_DIM], F32, tag="ex2", name="ex2", bufs=1)
                nc.vector.tensor_scalar(ex2, gst[:, MEM_DIM:512], 1.0 / n_total,
                                        None, op0=OP.mult)
                musq = work.tile([1, MEM_DIM], F32, tag="musq", name="musq", bufs=1)
                nc.vector.tensor_mul(musq, mu, mu)
                var = work.tile([1, MEM_DIM], F32, tag="var", name="var", bufs=1)
                nc.vector.tensor_sub(var, ex2, musq)
                sd = work.tile([1, MEM_DIM], F32, tag="sd", name="sd", bufs=1)
                nc.scalar.activation(sd, var, AF.Sqrt, bias=epsap)
                isd = work.tile([1, MEM_DIM], F32, tag="isd", name="isd", bufs=1)
                nc.vector.reciprocal(isd, sd)
                nc.vector.tensor_mul(a_ap, gamma_sb, isd)
                mua = work.tile([1, MEM_DIM], F32, tag="mua", name="mua", bufs=1)
                nc.vector.tensor_mul(mua, mu, a_ap)
                nc.vector.tensor_sub(b_ap, beta_sb, mua)

                if L == 1:
                    # per-partition (transposed-layout) affine params
                    for k in range(2):
                        for src, dstp in ((a_ap, aT[k]), (b_ap, bT[k])):
                            tp = psum_tp.tile([128, 1], F32, tag="tp2")
                            PE.transpose(
                                tp, src[:, k * 128:(k + 1) * 128], one_1x1)
                            nc.scalar.copy(dstp, tp)
                else:
                    # broadcast across partitions (row-layout affine)
                    bc = psum_sim.tile([128, 512], F32, tag="sim")
                    PE.matmul(bc, ones_row, ab, start=True, stop=True)
                    nc.scalar.copy(a2b, bc[:, 0:MEM_DIM])
                    nc.scalar.copy(b2b, bc[:, MEM_DIM:512])

            # L1 row-norm prologue: second read of x during the prep
            # window computes every tile's 1/||x|| in one batched chain,
            # so the steady-state DVE queue holds only topk+weights work.
            invn_all = consts.tile([128, nt], F32, name="invn_all")
            xns_all = consts.tile([128, nt], F32, name="xns_all")
            prep_bank(1)
            for i in range(nt):
                xpr = work.tile([128, MEM_DIM], F32, tag="xpr", name="xpr",
                                bufs=3)
                nc.sync.dma_start(xpr, x_d[i * 128:(i + 1) * 128, :])
                xsq = work.tile([128, MEM_DIM], F32, tag="sqs", name="sqs",
                                bufs=2)
                nc.vector.scalar_tensor_tensor(
                    out=xsq, in0=xpr, scalar=0.0, in1=xpr,
                    op0=OP.add, op1=OP.mult, accum_out=xns_all[:, i:i + 1])
            rsqrt_dve(invn_all, xns_all, iters=3, seed="recip")
            s1 = layer(1)
            prep_bank(2)  # fills the AllReduce bubble + L1 tail
            bn_allreduce(1, s1)
            bn_allreduce(2, layer(2))

            # ---- final: BN2 apply + leaky (Prelu) + store out ----
            # process TG row-tiles per wide op: fewer instructions and
            # semaphore round-trips in this PE-less serial tail
            TG = 2
            a2w = consts.tile([128, TG * MEM_DIM], F32, name="a2w")
            b2w = consts.tile([128, TG * MEM_DIM], F32, name="b2w")
            for q in range(TG):
                nc.scalar.copy(a2w[:, q * MEM_DIM:(q + 1) * MEM_DIM], a2b)
                nc.scalar.copy(b2w[:, q * MEM_DIM:(q + 1) * MEM_DIM], b2b)
            for i in range(0, nt, TG):
                hw_t = work.tile([128, TG * MEM_DIM], BF16, tag="h2i",
                                 name="h2i", bufs=3)
                for q in range(TG):
                    nc.sync.dma_start(
                        hw_t[:, q * MEM_DIM:(q + 1) * MEM_DIM],
                        h2_dram[(i + q) * 128:(i + q + 1) * 128, :])
                y = work.tile([128, TG * MEM_DIM], F32, tag="y", name="y", bufs=2)
                nc.vector.tensor_mul(y, hw_t, a2w)
                nc.vector.tensor_add(y, y, b2w)
                yo = work.tile([128, TG * MEM_DIM], F32, tag="yo", name="yo",
                               bufs=2)
                nc.scalar.activation(yo, y, AF.Prelu, alpha=LEAKY)
                for q in range(TG):
                    nc.sync.dma_start(
                        out_d[(i + q) * 128:(i + q + 1) * 128, :],
                        yo[:, q * MEM_DIM:(q + 1) * MEM_DIM])

    nc.compile()
    return nc


_CACHE = {}


def _get_nc(n_cores, rows_per_core, use_f32r=True):
    key = (n_cores, rows_per_core, use_f32r)
    if key not in _CACHE:
        _CACHE[key] = build_nc(n_cores, rows_per_core, use_f32r)
    return _CACHE[key]


def kernel(x, mem1, mem2, gamma1, beta1, gamma2, beta2, _trace=False,
           _use_f32r=True, _n_cores=8):
    n_cores = _n_cores
    n, d = x.shape
    rows_per_core = n // n_cores
    nc = _get_nc(n_cores, rows_per_core, _use_f32r)

    in_maps = []
    for c in range(n_cores):
        in_maps.append({
            "x": np.ascontiguousarray(x[c * rows_per_core:(c + 1) * rows_per_core]),
            "mem1": np.ascontiguousarray(mem1),
            "mem2": np.ascontiguousarray(mem2),
            "gamma1": np.ascontiguousarray(gamma1.reshape(1, -1)),
            "beta1": np.ascontiguousarray(beta1.reshape(1, -1)),
            "gamma2": np.ascontiguousarray(gamma2.reshape(1, -1)),
            "beta2": np.ascontiguousarray(beta2.reshape(1, -1)),
        })
    res = run_bass_kernel_spmd(nc, in_maps, list(range(n_cores)), trace=_trace)
    out = np.concatenate([res.results[c]["out"] for c in range(n_cores)], axis=0)
    if _trace:
        return out, res
    return out


# revision 33
# speedup vs baseline: 1.1159x; 1.0634x over previous
"""Trainium2 Bass kernel for nn_CMmodel (retrieval_knn).

Model (per layer, x2):
    sim = cosine(x, mem)                       # [N, 2048]
    S, I = top_k(sim, 10); w = softmax(relu(S))
    h = sum_k w[n,k] * mem[I[n,k]]             # [N, 256]
    h = leaky_relu(batchnorm(h))               # batch stats over ALL N rows

Strategy (8 cores, data-parallel over N):
  - Shard x rows 8 ways; replicate mem banks + BN params.
  - sim via PE matmul, 3-pass f32r exactness scheme (top-10 selection is
    precision-critical: a 10/11 rank swap is a ~50% error on that row, and
    even 1e-6 sim noise swaps ~30 rows of 32k).
  - Exact top-10 threshold t via DVE max8 + match_replace + max8.
  - U = (s>=t)*exp(s-t) via fused DVE scalar_tensor_tensor w/ accum Z.
  - h = U @ mem via PE: U transposed 128x128 on PE (fp32 for layer 1 since
    h1 feeds layer-2 sims; f32r for layer 2), matmul vs raw mem.
  - BatchNorm batch stats via ones-matmul into PSUM, deferred one tile so
    the PE never waits on the ACT/DVE drain chain; AllReduce'd (2KB).
  - ALL activations live in the 'natural_log_exp_and_others' ACT table set
    (Exp, Ln, Prelu, Copy, Square): rsqrt = exp(-0.5*ln), leaky = Prelu.
    Zero table swaps in steady state.
  - mem2 bank prep is emitted after layer 1 so it fills the AllReduce
    bubble; h2 stays SBUF-resident for the final BN2 apply.
"""
import sys

sys.path.insert(0, "/opt/trn_rl_repo")

import numpy as np

import concourse.bacc as bacc
import concourse.mybir as mybir
import concourse.tile as tile
from concourse.bass_utils import run_bass_kernel_spmd
from concourse.masks import make_identity
from concourse.tile import add_dep_helper

F32 = mybir.dt.float32
F32R = mybir.dt.float32r
BF16 = mybir.dt.bfloat16
AF = mybir.ActivationFunctionType
OP = mybir.AluOpType

MEM_DIM = 256
MEM_SIZE = 2048
K_TOP = 10
BN_EPS = 1e-5
LEAKY = 0.01

NJ = MEM_SIZE // 128  # 16 mem-row chunks
NEG_BIG = -1e30


def build_nc(n_cores: int, rows_per_core: int, use_f32r: bool = True):
    """Build the per-core Bass program (SPMD: same program all cores)."""
    nt = rows_per_core // 128  # x tiles per core
    n_total = rows_per_core * n_cores
    MMDT = F32R if use_f32r else F32
    nc = bacc.Bacc("TRN2", target_bir_lowering=False, debug=False,
                   num_devices=n_cores)

    x_d = nc.dram_tensor("x", [rows_per_core, MEM_DIM], F32, kind="ExternalInput")
    mem_d = {
        1: nc.dram_tensor("mem1", [MEM_SIZE, MEM_DIM], F32, kind="ExternalInput"),
        2: nc.dram_tensor("mem2", [MEM_SIZE, MEM_DIM], F32, kind="ExternalInput"),
    }
    gam_d = {
        1: nc.dram_tensor("gamma1", [1, MEM_DIM], F32, kind="ExternalInput"),
        2: nc.dram_tensor("gamma2", [1, MEM_DIM], F32, kind="ExternalInput"),
    }
    bet_d = {
        1: nc.dram_tensor("beta1", [1, MEM_DIM], F32, kind="ExternalInput"),
        2: nc.dram_tensor("beta2", [1, MEM_DIM], F32, kind="ExternalInput"),
    }
    out_d = nc.dram_tensor("out", [rows_per_core, MEM_DIM], F32, kind="ExternalOutput")

    with tile.TileContext(nc) as tc:
        with tc.tile_pool(name="consts", bufs=1) as consts, \
             tc.tile_pool(name="banks", bufs=1) as banks, \
             tc.tile_pool(name="work", bufs=1) as work, \
             tc.tile_pool(name="psum_sim", bufs=2, space="PSUM") as psum_sim, \
             tc.tile_pool(name="psum_tp", bufs=3, space="PSUM") as psum_tp, \
             tc.tile_pool(name="psum_h", bufs=2, space="PSUM") as psum_h_pool, \
             tc.tile_pool(name="psum_st", bufs=1, space="PSUM") as psum_st, \
             tc.tile_pool(name="dram", bufs=1, space="DRAM") as dram:

            # PE emission-order chain: accumulation groups must stay
            # contiguous on PE (interleaved matmuls drop accumulates).
            class _PEChain:
                def __init__(self):
                    self.last = None

                def _chain(self, binst):
                    if self.last is not None:
                        add_dep_helper(binst.ins, self.last.ins, sync=False,
                                       reason="pe-order")
                    self.last = binst
                    return binst

                def matmul(self, *a, **kw):
                    return self._chain(nc.tensor.matmul(*a, **kw))

                def transpose(self, *a, **kw):
                    return self._chain(nc.tensor.transpose(*a, **kw))

            PE = _PEChain()

            # ---------------- constants ----------------
            ident = consts.tile([128, 128], F32)
            make_identity(nc, ident)
            ident_b = consts.tile([128, 128], BF16)
            nc.scalar.copy(ident_b, ident)  # exact: 0/1 values
            ones_col = consts.tile([128, 1], F32)
            nc.vector.memset(ones_col, 1.0)
            ones_col_b = consts.tile([128, 1], BF16)
            nc.vector.memset(ones_col_b, 1.0)
            one_1x1 = consts.tile([1, 1], F32)
            nc.vector.memset(one_1x1, 1.0)
            ones_row = consts.tile([1, 128], F32)
            nc.vector.memset(ones_row, 1.0)
            epsap = consts.tile([1, 1], F32)
            nc.vector.memset(epsap, BN_EPS)

            gb = {}
            for L in (1, 2):
                g = consts.tile([1, MEM_DIM], F32, name=f"gamma_sb{L}")
                b = consts.tile([1, MEM_DIM], F32, name=f"beta_sb{L}")
                nc.sync.dma_start(g, gam_d[L][:])
                nc.sync.dma_start(b, bet_d[L][:])
                gb[L] = (g, b)

            # ---------------- mem banks (prep emitted lazily) ----------------
            # mraw_b[L]: raw mem, natural layout [128, NJ*256] (rhs of h mm)
            # mnT[L,k] : row-normalized mem, transposed, f32r-rounded
            # mnTres   : bf16 residual (m/||m|| - round(m/||m||))
            mraw_b = {}
            mnT = {}
            mnTres = {}
            for L in (1, 2):
                # L1 h-matmul must be fp32-exact (h1 feeds layer-2 sims);
                # L2's only feeds the final output, so bf16 is plenty.
                mraw_b[L] = banks.tile([128, NJ * MEM_DIM],
                                       F32 if L == 1 else BF16, name=f"mraw{L}")
                mnT[L] = [
                    banks.tile([128, MEM_SIZE], MMDT, name=f"mnT{L}_{k}")
                    for k in range(2)
                ]
                mnTres[L] = [
                    banks.tile([128, MEM_SIZE], BF16, name=f"mnTres{L}_{k}")
                    for k in range(2)
                ]

            def prep_bank(L):
                """Load + normalize + transpose one memory bank.

                Batch the norm chain across all 16 chunks (one Ln+Exp rsqrt
                + one batched Newton refine), then per-chunk scale + PE
                transposes. L1 DMAs straight into its fp32 bank; L2's bank
                is bf16, so the fp32 rows are staged (DMA'd twice - once
                for norms, once for scale+convert) through work tiles.
                """
                def stage_in(j, bufs=2):
                    if L == 1:
                        return mraw_b[1][:, j * MEM_DIM:(j + 1) * MEM_DIM]
                    stg = work.tile([128, MEM_DIM], F32, tag="mstg",
                                    name="mstg", bufs=bufs)
                    nc.sync.dma_start(stg, mem_d[L][j * 128:(j + 1) * 128, :])
                    return stg

                if L == 1:
                    for j in range(NJ):
                        nc.sync.dma_start(
                            mraw_b[1][:, j * MEM_DIM:(j + 1) * MEM_DIM],
                            mem_d[1][j * 128:(j + 1) * 128, :])
                mns = work.tile([128, NJ], F32, tag="mns", name="mns", bufs=1)
                for j in range(NJ):
                    src = stage_in(j)
                    msq = work.tile([128, MEM_DIM], F32, tag="sqs", name="sqs",
                                    bufs=2)
                    nc.scalar.activation(msq, src, AF.Square,
                                         accum_out=mns[:, j:j + 1])
                # batched rsqrt, all on DVE (mem-norm precision reorders
                # near-tied sims; 2 Newton steps make it fp32-exact)
                inm = work.tile([128, NJ], F32, tag="inm", name="inm", bufs=1)
                rsqrt_dve(inm, mns, iters=4, seed=1.73)
                for j in range(NJ):
                    src = stage_in(j)
                    if L == 2:  # bf16 bank copy for the h2 matmul rhs
                        nc.vector.tensor_copy(
                            mraw_b[2][:, j * MEM_DIM:(j + 1) * MEM_DIM], src)
                    mnsc = work.tile([128, MEM_DIM], F32, tag="mnsc", name="mnsc",
                                     bufs=2)
                    nc.scalar.mul(mnsc, src, inm[:, j:j + 1])
                    for k in range(2):
                        tp = psum_tp.tile([128, 128], F32, tag="tp2")
                        PE.transpose(tp, mnsc[:, k * 128:(k + 1) * 128], ident)
                        dstT = mnT[L][k][:, j * 128:(j + 1) * 128]
                        nc.scalar.copy(dstT, tp)                   # f32r round
                        # bf16 residual straight from psum - rounded
                        nc.vector.tensor_sub(
                            mnTres[L][k][:, j * 128:(j + 1) * 128],
                            tp, dstT.bitcast(F32))

            # ---------------- persistent stores ----------------
            # h1 and h2 both spill to DRAM (SBUF goes to pipeline buffers)
            h1_dram = nc.dram_tensor("h1buf", [rows_per_core, MEM_DIM], F32)
            h2_dram = nc.dram_tensor("h2buf", [rows_per_core, MEM_DIM], BF16)
            # BN affine params (filled after each AllReduce)
            aT = [consts.tile([128, 1], F32, name=f"aT{k}") for k in range(2)]
            bT = [consts.tile([128, 1], F32, name=f"bT{k}") for k in range(2)]
            a2b = consts.tile([128, MEM_DIM], F32, name="a2b")
            b2b = consts.tile([128, MEM_DIM], F32, name="b2b")

            def rsqrt_dve(dst, src, iters=3, seed="recip", gp=False):
                """dst = 1/sqrt(src), entirely on DVE, keeping Sqrt off the
                ACT engine so the Exp table never gets evicted (each table
                load costs 1.3us). Seed: either a float constant, or
                "recip" = secant fit a*(1/x)+b of sqrt(1/x) over x in
                [60, 400] (max 10% off). Newton steps y*(1.5-0.5*x*y^2)
                converge regardless: 3 iters from 10% -> ~2e-7 rel."""
                shp = list(src.shape)
                eng = nc.gpsimd if gp else nc.vector
                y = work.tile(shp, F32, tag="ny0", name="ny0", bufs=2)
                if seed == "recip":
                    rr = work.tile(shp, F32, tag="nrr", name="nrr", bufs=2)
                    nc.vector.reciprocal(rr, src)
                    eng.tensor_scalar(y, rr, 5.56, 0.0361,
                                      op0=OP.mult, op1=OP.add)
                else:
                    eng.tensor_scalar(y, src, 0.0, float(seed),
                                      op0=OP.mult, op1=OP.add)
                for it in range(iters):
                    t1 = work.tile(shp, F32, tag="nt2", name="nt2", bufs=2)
                    eng.tensor_mul(t1, y, y)
                    eng.tensor_mul(t1, t1, src)
                    eng.tensor_scalar(t1, t1, -0.5, 1.5, op0=OP.mult,
                                      op1=OP.add)
                    out = dst if it == iters - 1 else work.tile(
                        shp, F32, tag=f"ny{it + 1}", name=f"ny{it + 1}", bufs=2)
                    eng.tensor_mul(out, y, t1)
                    y = out

            def stage1a(L, i):
                """lhsT prep for tile i: DMA + transposes + f32r/bf16 splits
                + row-norm rsqrt. Emitted one tile AHEAD of stage1b(i-1)'s
                sims so the ACT queue serves these copies before that tile's
                exp/uts work (otherwise the PE stalls on LDWEIGHTS).
                Small elementwise goes to the idle GpSimd engine."""
                lhsT_f = [
                    work.tile([128, 128], F32, tag=f"lhsTf{k}", name=f"lhsTf{k}", bufs=2)
                    for k in range(2)
                ]
                lhsT_r = [
                    work.tile([128, 128], MMDT, tag=f"lhsTr{k}", name=f"lhsTr{k}", bufs=2)
                    for k in range(2)
                ]
                lhsT_s = [
                    work.tile([128, 128], MMDT, tag=f"lhsTs{k}", name=f"lhsTs{k}", bufs=2)
                    for k in range(2)
                ]
                lhsT_rb = [
                    work.tile([128, 128], BF16, tag=f"lhsTb{k}", name=f"lhsTb{k}", bufs=2)
                    for k in range(2)
                ]
                if L == 1:
                    invn = invn_all[:, i:i + 1]
                    xi = work.tile([128, MEM_DIM], F32, tag="xi", name="xi", bufs=3)
                    nc.sync.dma_start(xi, x_d[i * 128:(i + 1) * 128, :])
                    tpx = psum_tp.tile([128, 256], F32, tag="tp2")
                    for k in range(2):
                        PE.transpose(tpx[:, k * 128:(k + 1) * 128],
                                     xi[:, k * 128:(k + 1) * 128], ident)
                    for k in range(2):
                        nc.scalar.copy(lhsT_f[k], tpx[:, k * 128:(k + 1) * 128])
                else:
                    ns_ps = psum_tp.tile([1, 128], F32, tag="tp2")
                    hsl = work.tile([128, MEM_DIM], F32, tag="h1i", name="h1i", bufs=3)
                    nc.sync.dma_start(hsl, h1_dram[i * 128:(i + 1) * 128, :])
                    sqTs = []
                    tph = psum_tp.tile([128, 256], F32, tag="tp2")
                    for k in range(2):
                        PE.transpose(tph[:, k * 128:(k + 1) * 128],
                                     hsl[:, k * 128:(k + 1) * 128], ident)
                    for k in range(2):
                        # fused BN1 apply + leaky relu at the drain (Prelu
                        # shares the Exp ACT table set; Lrelu doesn't)
                        nc.scalar.activation(
                            lhsT_f[k], tph[:, k * 128:(k + 1) * 128], AF.Prelu,
                            bias=bT[k], scale=aT[k], alpha=LEAKY,
                        )
                        sqT = work.tile([128, 128], F32, tag=f"sqT{k}", name=f"sqT{k}", bufs=2)
                        nc.scalar.activation(sqT, lhsT_f[k], AF.Square)
                        sqTs.append(sqT)
                    for k in range(2):
                        PE.matmul(ns_ps, ones_col, sqTs[k],
                                  start=(k == 0), stop=(k == 1))
                    ns_sb = work.tile([1, 128], F32, tag="ns_sb", name="ns_sb", bufs=2)
                    nc.scalar.copy(ns_sb, ns_ps)
                    nsT = psum_tp.tile([128, 1], F32, tag="tp2")
                    PE.transpose(nsT, ns_sb, one_1x1)
                    invn = work.tile([128, 1], F32, tag="invn", name="invn",
                                     bufs=2)
                    nsS = work.tile([128, 1], F32, tag="nsS", name="nsS", bufs=2)
                    nc.scalar.copy(nsS, nsT)
                    # the rsqrt chain is emitted LATER (after stage1b of the
                    # previous tile) so the DVE queue serves topk first;
                    # invn is only needed at the exp, well past topk.
                for k in range(2):
                    nc.scalar.copy(lhsT_r[k], lhsT_f[k])   # f32r round
                    nc.vector.tensor_copy(lhsT_rb[k], lhsT_f[k])  # bf16 (pass C)
                    rsd = work.tile([128, 128], F32, tag="rsd", name="rsd", bufs=2)
                    nc.vector.tensor_sub(rsd, lhsT_f[k], lhsT_r[k].bitcast(F32))
                    nc.scalar.copy(lhsT_s[k], rsd)         # f32r residual
                return dict(lhsT_r=lhsT_r, lhsT_s=lhsT_s, lhsT_rb=lhsT_rb,
                            invn=invn, nsS=None if L == 1 else nsS)

            def emit_invn(sa):
                if sa is not None and sa.get("nsS") is not None:
                    rsqrt_dve(sa["invn"], sa["nsS"])

            def stage1b(L, i, sa):
                """3-pass sim matmul + top-10 threshold + masked exp
                weights for tile i (lhsT prepped by stage1a)."""
                lhsT_r, lhsT_s = sa["lhsT_r"], sa["lhsT_s"]
                lhsT_rb, invn = sa["lhsT_rb"], sa["invn"]
                # 3-pass f32r sim matmul: r(x)@r(m) + res_x@r(m) + x_b@res_m_b
                s_sb = work.tile([128, MEM_SIZE], F32, tag="s_sb", name="s_sb", bufs=2)
                for c in range(4):
                    ps = psum_sim.tile([128, 512], F32, tag="sim")
                    for k in range(2):
                        PE.matmul(ps, lhsT_r[k],
                                  mnT[L][k][:, c * 512:(c + 1) * 512],
                                  start=(k == 0), stop=False)
                    for k in range(2):
                        PE.matmul(ps, lhsT_s[k],
                                  mnT[L][k][:, c * 512:(c + 1) * 512],
                                  start=False, stop=False)
                    for k in range(2):
                        PE.matmul(ps, lhsT_rb[k],
                                  mnTres[L][k][:, c * 512:(c + 1) * 512],
                                  start=False, stop=(k == 1))
                    nc.scalar.copy(s_sb[:, c * 512:(c + 1) * 512], ps)

                # exact 10th-largest threshold
                m8a = work.tile([128, 8], F32, tag="m8a", name="m8a", bufs=2)
                nc.vector.max(out=m8a, in_=s_sb)
                s_z = work.tile([128, MEM_SIZE], F32, tag="s_z", name="s_z", bufs=2)
                nc.vector.match_replace(out=s_z, in_to_replace=m8a,
                                        in_values=s_sb, imm_value=NEG_BIG)
                m8b = work.tile([128, 8], F32, tag="m8b", name="m8b", bufs=2)
                nc.vector.max(out=m8b, in_=s_z)
                t_ap = m8b[:, K_TOP - 8 - 1:K_TOP - 8]  # 10th largest
                neg_t = work.tile([128, 1], F32, tag="neg_t", name="neg_t", bufs=2)
                nc.vector.tensor_scalar(neg_t, t_ap, invn, -1.0,
                                        op0=OP.mult, op1=OP.mult)

                # shifted exp weights, masked, with sum
                e = work.tile([128, MEM_SIZE], F32, tag="e", name="e", bufs=1)
                nc.scalar.activation(e, s_sb, AF.Exp, bias=neg_t, scale=invn)
                if L == 1:
                    U = s_z  # reuse buffer: s_z is dead after m8b
                else:
                    # bf16 weights: DVE rounds on write, so the PE's bf16
                    # fast path (1 cy/row transposes + h2 matmul) is legal
                    U = work.tile([128, MEM_SIZE], BF16, tag="Ub", name="Ub",
                                  bufs=2)
                Z = work.tile([128, 1], F32, tag="Z", name="Z", bufs=2)
                nc.vector.scalar_tensor_tensor(
                    out=U, in0=s_sb, scalar=t_ap, in1=e,
                    op0=OP.is_ge, op1=OP.mult, accum_out=Z,
                )
                rz = work.tile([128, 1], F32, tag="rz", name="rz", bufs=2)
                nc.vector.reciprocal(rz, Z)
                return dict(U=U, rz=rz)

            def stage2(L, i, st, stats_acc, pend):
                """U transposes + h = (U/Z) @ mem + BN batch-stat partials.

                The stats matmuls for THIS tile are deferred: they're pushed
                on `pend` and emitted by the NEXT stage2 call (or the flush),
                so the PE never waits on the dst/sqh drain chain.
                """
                U, rz = st["U"], st["rz"]
                # layer 1 feeds layer-2 sims: must stay fp32-exact.
                # layer 2 only feeds the final output: bf16 is plenty and
                # transposes at 1 cy/row instead of fp32's 2.5.
                ut_dt = F32 if L == 1 else BF16
                tp_ident = ident if L == 1 else ident_b
                # paired transposes -> one [128,256] drain per pair
                uts = []
                for c2 in range(NJ // 2):
                    tp2 = psum_tp.tile([128, 256], ut_dt, tag="tp2")
                    PE.transpose(tp2[:, 0:128],
                                 U[:, (2 * c2) * 128:(2 * c2 + 1) * 128], tp_ident)
                    PE.transpose(tp2[:, 128:256],
                                 U[:, (2 * c2 + 1) * 128:(2 * c2 + 2) * 128], tp_ident)
                    utp = work.tile([128, 256], ut_dt, tag="ut", name="ut",
                                    bufs=NJ // 2 + 2)
                    nc.scalar.copy(utp, tp2)
                    uts.append(utp)
                hp = psum_h_pool.tile([128, MEM_DIM], F32, tag="hp")
                for c in range(NJ):
                    PE.matmul(
                        hp, uts[c // 2][:, (c % 2) * 128:(c % 2 + 1) * 128],
                        mraw_b[L][:, c * MEM_DIM:(c + 1) * MEM_DIM],
                        start=(c == 0), stop=(c == NJ - 1),
                    )
                # drain h (normalized by Z) + square; stats matmuls deferred.
                # h2 is drained + spilled + stat'd in bf16 (halves the tail
                # DMA; BN2 normalizes the rounded values consistently).
                dst = work.tile([128, MEM_DIM], F32 if L == 1 else BF16,
                                tag="h2o", name="h2o", bufs=4)
                nc.scalar.mul(dst, hp, rz)
                h_dram = h1_dram if L == 1 else h2_dram
                nc.sync.dma_start(h_dram[i * 128:(i + 1) * 128, :], dst)
                sqh = work.tile([128, MEM_DIM], F32, tag="sqh", name="sqh", bufs=3)
                nc.vector.tensor_mul(sqh, dst, dst)
                emit_pending_stats(L, stats_acc, pend)
                pend.append((dst, sqh))

            def emit_pending_stats(L, stats_acc, pend):
                ones_l = ones_col if L == 1 else ones_col_b
                while pend:
                    dst, sqh = pend.pop(0)
                    pd = psum_st.tile([1, 512], F32, tag="st")
                    PE.matmul(pd[:, 0:MEM_DIM], ones_l, dst,
                              start=True, stop=True)
                    PE.matmul(pd[:, MEM_DIM:2 * MEM_DIM], ones_col, sqh,
                              start=True, stop=True)
                    nc.vector.tensor_add(stats_acc, stats_acc, pd)

            def layer(L):
                stats_acc = work.tile([1, 512], F32, tag=f"stacc{L}", bufs=1,
                                      name=f"stats_acc{L}")
                nc.vector.memset(stats_acc, 0.0)
                pend = []
                prev = None
                sa = stage1a(L, 0)
                emit_invn(sa)
                for i in range(nt):
                    sa_next = stage1a(L, i + 1) if i + 1 < nt else None
                    st = stage1b(L, i, sa)
                    emit_invn(sa_next)
                    sa = sa_next
                    if prev is not None:
                        stage2(L, i - 1, prev, stats_acc, pend)
                    prev = st
                stage2(L, nt - 1, prev, stats_acc, pend)
                emit_pending_stats(L, stats_acc, pend)
                return stats_acc

            def bn_allreduce(L, stats_acc):
                gamma_sb, beta_sb = gb[L]
                stats_sb = stats_acc
                ar_in = dram.tile([1, 512], F32, name=f"ar_in{L}")
                ar_out = dram.tile([1, 512], F32, addr_space="Shared",
                                   name=f"ar_out{L}")
                nc.sync.dma_start(ar_in, stats_sb)
                nc.gpsimd.collective_compute(
                    "AllReduce", OP.add,
                    replica_groups=[list(range(n_cores))],
                    ins=[ar_in[:]], outs=[ar_out[:]],
                )
                gst = work.tile([1, 512], F32, tag="gst", name="gst", bufs=1)
                nc.sync.dma_start(gst, ar_out)

                ab = work.tile([1, 512], F32, tag="ab", name="ab", bufs=1)
                a_ap, b_ap = ab[:, 0:MEM_DIM], ab[:, MEM_DIM:512]
                mu = work.tile([1, MEM_DIM], F32, tag="mu", name="mu", bufs=1)
                nc.vector.tensor_scalar(mu, gst[:, 0:MEM_DIM], 1.0 / n_total,
                                        None, op0=OP.mult)
                ex2 = work.tile([1, MEM_DIM], F32, tag="ex2", name="ex2", bufs=1)
                nc.vector.tensor_scalar(ex2, gst[:, MEM_DIM:512], 1.0 / n_total,
                                        None, op0=OP.mult)
                musq = work.tile([1, MEM_DIM], F32, tag="musq", name="musq", bufs=1)
                nc.vector.tensor_mul(musq, mu, mu)
                var = work.tile([1, MEM_DIM], F32, tag="var", name="var", bufs=1)
                nc.vector.tensor_sub(var, ex2, musq)
                sd = work.tile([1, MEM_DIM], F32, tag="sd", name="sd", bufs=1)
                nc.scalar.activation(sd, var, AF.Sqrt, bias=epsap)
                isd = work.tile([1, MEM_DIM], F32, tag="isd", name="isd", bufs=1)
                nc.vector.reciprocal(isd, sd)
                nc.vector.tensor_mul(a_ap, gamma_sb, isd)
                mua = work.tile([1, MEM_DIM], F32, tag="mua", name="mua", bufs=1)
                nc.vector.tensor_mul(mua, mu, a_ap)
                nc.vector.tensor_sub(b_ap, beta_sb, mua)

                if L == 1:
                    # per-partition (transposed-layout) affine params
                    for k in range(2):
                        for src, dstp in ((a_ap, aT[k]), (b_ap, bT[k])):
                            tp = psum_tp.tile([128, 1], F32, tag="tp2")
                            PE.transpose(
                                tp, src[:, k * 128:(k + 1) * 128], one_1x1)
                            nc.scalar.copy(dstp, tp)
                else:
                    # broadcast across partitions (row-layout affine)
                    bc = psum_sim.tile([128, 512], F32, tag="sim")
                    PE.matmul(bc, ones_row, ab, start=True, stop=True)
                    nc.scalar.copy(a2b, bc[:, 0:MEM_DIM])
                    nc.scalar.copy(b2b, bc[:, MEM_DIM:512])

            # L1 row-norm prologue: second read of x during the prep
            # window computes every tile's 1/||x|| in one batched chain,
            # so the steady-state DVE queue holds only topk+weights work.
            invn_all = consts.tile([128, nt], F32, name="invn_all")
            xns_all = consts.tile([128, nt], F32, name="xns_all")
            prep_bank(1)
            for i in range(nt):
                xpr = work.tile([128, MEM_DIM], F32, tag="xpr", name="xpr",
                                bufs=3)
                nc.sync.dma_start(xpr, x_d[i * 128:(i + 1) * 128, :])
                xsq = work.tile([128, MEM_DIM], F32, tag="sqs", name="sqs",
                                bufs=2)
                nc.vector.scalar_tensor_tensor(
                    out=xsq, in0=xpr, scalar=0.0, in1=xpr,
                    op0=OP.add, op1=OP.mult, accum_out=xns_all[:, i:i + 1])
            rsqrt_dve(invn_all, xns_all, iters=3, seed="recip")
            s1 = layer(1)
            prep_bank(2)  # fills the AllReduce bubble + L1 tail
            bn_allreduce(1, s1)
            bn_allreduce(2, layer(2))

            # ---- final: BN2 apply + leaky (Prelu) + store out ----
            # process TG row-tiles per wide op: fewer instructions and
            # semaphore round-trips in this PE-less serial tail
            TG = 2
            a2w = consts.tile([128, TG * MEM_DIM], F32, name="a2w")
            b2w = consts.tile([128, TG * MEM_DIM], F32, name="b2w")
            for q in range(TG):
                nc.scalar.copy(a2w[:, q * MEM_DIM:(q + 1) * MEM_DIM], a2b)
                nc.scalar.copy(b2w[:, q * MEM_DIM:(q + 1) * MEM_DIM], b2b)
            for i in range(0, nt, TG):
                hw_t = work.tile([128, TG * MEM_DIM], BF16, tag="h2i",
                                 name="h2i", bufs=3)
                for q in range(TG):
                    nc.sync.dma_start(
                        hw_t[:, q * MEM_DIM:(q + 1) * MEM_DIM],
                        h2_dram[(i + q) * 128:(i + q + 1) * 128, :])
                y = work.tile([128, TG * MEM_DIM], F32, tag="y", name="y", bufs=2)
                nc.vector.tensor_mul(y, hw_t, a2w)
                nc.vector.tensor_add(y, y, b2w)
                yo = work.tile([128, TG * MEM_DIM], F32, tag="yo", name="yo",
                               bufs=2)
                nc.scalar.activation(yo, y, AF.Prelu, alpha=LEAKY)
                for q in range(TG):
                    nc.sync.dma_start(
                        out_d[(i + q) * 128:(i + q + 1) * 128, :],
                        yo[:, q * MEM_DIM:(q + 1) * MEM_DIM])

    nc.compile()
    return nc


_CACHE = {}


def _get_nc(n_cores, rows_per_core, use_f32r=True):
    key = (n_cores, rows_per_core, use_f32r)
    if key not in _CACHE:
        _CACHE[key] = build_nc(n_cores, rows_per_core, use_f32r)
    return _CACHE[key]


def kernel(x, mem1, mem2, gamma1, beta1, gamma2, beta2, _trace=False,
           _use_f32r=True, _n_cores=8):
    n_cores = _n_cores
    n, d = x.shape
    rows_per_core = n // n_cores
    nc = _get_nc(n_cores, rows_per_core, _use_f32r)

    in_maps = []
    for c in range(n_cores):
        in_maps.append({
            "x": np.ascontiguousarray(x[c * rows_per_core:(c + 1) * rows_per_core]),
            "mem1": np.ascontiguousarray(mem1),
            "mem2": np.ascontiguousarray(mem2),
            "gamma1": np.ascontiguousarray(gamma1.reshape(1, -1)),
            "beta1": np.ascontiguousarray(beta1.reshape(1, -1)),
            "gamma2": np.ascontiguousarray(gamma2.reshape(1, -1)),
            "beta2": np.ascontiguousarray(beta2.reshape(1, -1)),
        })
    res = run_bass_kernel_spmd(nc, in_maps, list(range(n_cores)), trace=_trace)
    out = np.concatenate([res.results[c]["out"] for c in range(n_cores)], axis=0)
    if _trace:
        return out, res
    return out


# revision 35
# speedup vs baseline: 1.1550x; 1.0350x over previous
"""Trainium2 Bass kernel for nn_CMmodel (retrieval_knn).

Model (per layer, x2):
    sim = cosine(x, mem)                       # [N, 2048]
    S, I = top_k(sim, 10); w = softmax(relu(S))
    h = sum_k w[n,k] * mem[I[n,k]]             # [N, 256]
    h = leaky_relu(batchnorm(h))               # batch stats over ALL N rows

Strategy (8 cores, data-parallel over N):
  - Shard x rows 8 ways; replicate mem banks + BN params.
  - sim via PE matmul, 3-pass f32r exactness scheme (top-10 selection is
    precision-critical: a 10/11 rank swap is a ~50% error on that row, and
    even 1e-6 sim noise swaps ~30 rows of 32k).
  - Exact top-10 threshold t via DVE max8 + match_replace + max8.
  - U = (s>=t)*exp(s-t) via fused DVE scalar_tensor_tensor w/ accum Z.
  - h = U @ mem via PE: U transposed 128x128 on PE (fp32 for layer 1 since
    h1 feeds layer-2 sims; f32r for layer 2), matmul vs raw mem.
  - BatchNorm batch stats via ones-matmul into PSUM, deferred one tile so
    the PE never waits on the ACT/DVE drain chain; AllReduce'd (2KB).
  - ALL activations live in the 'natural_log_exp_and_others' ACT table set
    (Exp, Ln, Prelu, Copy, Square): rsqrt = exp(-0.5*ln), leaky = Prelu.
    Zero table swaps in steady state.
  - mem2 bank prep is emitted after layer 1 so it fills the AllReduce
    bubble; h2 stays SBUF-resident for the final BN2 apply.
"""
import sys

sys.path.insert(0, "/opt/trn_rl_repo")

import numpy as np

import concourse.bacc as bacc
import concourse.mybir as mybir
import concourse.tile as tile
from concourse.bass_utils import run_bass_kernel_spmd
from concourse.masks import make_identity
from concourse.tile import add_dep_helper

F32 = mybir.dt.float32
F32R = mybir.dt.float32r
BF16 = mybir.dt.bfloat16
AF = mybir.ActivationFunctionType
OP = mybir.AluOpType

MEM_DIM = 256
MEM_SIZE = 2048
K_TOP = 10
BN_EPS = 1e-5
LEAKY = 0.01

NJ = MEM_SIZE // 128  # 16 mem-row chunks
NEG_BIG = -1e30


def build_nc(n_cores: int, rows_per_core: int, use_f32r: bool = True):
    """Build the per-core Bass program (SPMD: same program all cores)."""
    nt = rows_per_core // 128  # x tiles per core
    n_total = rows_per_core * n_cores
    MMDT = F32R if use_f32r else F32
    nc = bacc.Bacc("TRN2", target_bir_lowering=False, debug=False,
                   num_devices=n_cores)

    x_d = nc.dram_tensor("x", [rows_per_core, MEM_DIM], F32, kind="ExternalInput")
    mem_d = {
        1: nc.dram_tensor("mem1", [MEM_SIZE, MEM_DIM], F32, kind="ExternalInput"),
        2: nc.dram_tensor("mem2", [MEM_SIZE, MEM_DIM], F32, kind="ExternalInput"),
    }
    gam_d = {
        1: nc.dram_tensor("gamma1", [1, MEM_DIM], F32, kind="ExternalInput"),
        2: nc.dram_tensor("gamma2", [1, MEM_DIM], F32, kind="ExternalInput"),
    }
    bet_d = {
        1: nc.dram_tensor("beta1", [1, MEM_DIM], F32, kind="ExternalInput"),
        2: nc.dram_tensor("beta2", [1, MEM_DIM], F32, kind="ExternalInput"),
    }
    out_d = nc.dram_tensor("out", [rows_per_core, MEM_DIM], F32, kind="ExternalOutput")

    with tile.TileContext(nc) as tc:
        with tc.tile_pool(name="consts", bufs=1) as consts, \
             tc.tile_pool(name="banks", bufs=1) as banks, \
             tc.tile_pool(name="work", bufs=1) as work, \
             tc.tile_pool(name="psum_sim", bufs=2, space="PSUM") as psum_sim, \
             tc.tile_pool(name="psum_tp", bufs=3, space="PSUM") as psum_tp, \
             tc.tile_pool(name="psum_h", bufs=2, space="PSUM") as psum_h_pool, \
             tc.tile_pool(name="psum_st", bufs=1, space="PSUM") as psum_st, \
             tc.tile_pool(name="dram", bufs=1, space="DRAM") as dram:

            # PE emission-order chain: accumulation groups must stay
            # contiguous on PE (interleaved matmuls drop accumulates).
            class _PEChain:
                def __init__(self):
                    self.last = None

                def _chain(self, binst):
                    if self.last is not None:
                        add_dep_helper(binst.ins, self.last.ins, sync=False,
                                       reason="pe-order")
                    self.last = binst
                    return binst

                def matmul(self, *a, **kw):
                    return self._chain(nc.tensor.matmul(*a, **kw))

                def transpose(self, *a, **kw):
                    return self._chain(nc.tensor.transpose(*a, **kw))

            PE = _PEChain()

            # ---------------- constants ----------------
            ident = consts.tile([128, 128], F32)
            make_identity(nc, ident)
            ident_b = consts.tile([128, 128], BF16)
            nc.scalar.copy(ident_b, ident)  # exact: 0/1 values
            ones_col = consts.tile([128, 1], F32)
            nc.vector.memset(ones_col, 1.0)
            ones_col_b = consts.tile([128, 1], BF16)
            nc.vector.memset(ones_col_b, 1.0)
            one_1x1 = consts.tile([1, 1], F32)
            nc.vector.memset(one_1x1, 1.0)
            ones_row = consts.tile([1, 128], F32)
            nc.vector.memset(ones_row, 1.0)
            epsap = consts.tile([1, 1], F32)
            nc.vector.memset(epsap, BN_EPS)

            gb = {}
            for L in (1, 2):
                g = consts.tile([1, MEM_DIM], F32, name=f"gamma_sb{L}")
                b = consts.tile([1, MEM_DIM], F32, name=f"beta_sb{L}")
                nc.sync.dma_start(g, gam_d[L][:])
                nc.sync.dma_start(b, bet_d[L][:])
                gb[L] = (g, b)

            # ---------------- mem banks (prep emitted lazily) ----------------
            # mraw_b[L]: raw mem, natural layout [128, NJ*256] (rhs of h mm)
            # mnT[L,k] : row-normalized mem, transposed, f32r-rounded
            # mnTres   : bf16 residual (m/||m|| - round(m/||m||))
            mraw_b = {}
            mnT = {}
            mnTres = {}
            for L in (1, 2):
                # L1 h-matmul must be fp32-exact (h1 feeds layer-2 sims);
                # L2's only feeds the final output, so bf16 is plenty.
                mraw_b[L] = banks.tile([128, NJ * MEM_DIM],
                                       F32 if L == 1 else BF16, name=f"mraw{L}")
                mnT[L] = [
                    banks.tile([128, MEM_SIZE], MMDT, name=f"mnT{L}_{k}")
                    for k in range(2)
                ]
                mnTres[L] = [
                    banks.tile([128, MEM_SIZE], BF16, name=f"mnTres{L}_{k}")
                    for k in range(2)
                ]

            def prep_bank(L):
                """Load + normalize + transpose one memory bank.

                Batch the norm chain across all 16 chunks (one Ln+Exp rsqrt
                + one batched Newton refine), then per-chunk scale + PE
                transposes. L1 DMAs straight into its fp32 bank; L2's bank
                is bf16, so the fp32 rows are staged (DMA'd twice - once
                for norms, once for scale+convert) through work tiles.
                """
                def stage_in(j, bufs=2):
                    if L == 1:
                        return mraw_b[1][:, j * MEM_DIM:(j + 1) * MEM_DIM]
                    stg = work.tile([128, MEM_DIM], F32, tag="mstg",
                                    name="mstg", bufs=bufs)
                    nc.sync.dma_start(stg, mem_d[L][j * 128:(j + 1) * 128, :])
                    return stg

                if L == 1:
                    for j in range(NJ):
                        nc.sync.dma_start(
                            mraw_b[1][:, j * MEM_DIM:(j + 1) * MEM_DIM],
                            mem_d[1][j * 128:(j + 1) * 128, :])
                mns = work.tile([128, NJ], F32, tag="mns", name="mns", bufs=1)
                for j in range(NJ):
                    src = stage_in(j)
                    msq = work.tile([128, MEM_DIM], F32, tag="sqs", name="sqs",
                                    bufs=2)
                    nc.scalar.activation(msq, src, AF.Square,
                                         accum_out=mns[:, j:j + 1])
                # batched rsqrt, all on DVE (mem-norm precision reorders
                # near-tied sims; 2 Newton steps make it fp32-exact)
                inm = work.tile([128, NJ], F32, tag="inm", name="inm", bufs=1)
                rsqrt_dve(inm, mns, iters=4, seed=1.73)
                for j in range(NJ):
                    src = stage_in(j)
                    if L == 2:  # bf16 bank copy for the h2 matmul rhs
                        nc.vector.tensor_copy(
                            mraw_b[2][:, j * MEM_DIM:(j + 1) * MEM_DIM], src)
                    mnsc = work.tile([128, MEM_DIM], F32, tag="mnsc", name="mnsc",
                                     bufs=2)
                    nc.scalar.mul(mnsc, src, inm[:, j:j + 1])
                    for k in range(2):
                        tp = psum_tp.tile([128, 128], F32, tag="tp2")
                        PE.transpose(tp, mnsc[:, k * 128:(k + 1) * 128], ident)
                        dstT = mnT[L][k][:, j * 128:(j + 1) * 128]
                        nc.scalar.copy(dstT, tp)                   # f32r round
                        # bf16 residual straight from psum - rounded
                        nc.vector.tensor_sub(
                            mnTres[L][k][:, j * 128:(j + 1) * 128],
                            tp, dstT.bitcast(F32))

            # ---------------- persistent stores ----------------
            # h1 and h2 both spill to DRAM (SBUF goes to pipeline buffers)
            h1_dram = nc.dram_tensor("h1buf", [rows_per_core, MEM_DIM], F32)
            h2_dram = nc.dram_tensor("h2buf", [rows_per_core, MEM_DIM], BF16)
            # BN affine params (filled after each AllReduce)
            aT = [consts.tile([128, 1], F32, name=f"aT{k}") for k in range(2)]
            bT = [consts.tile([128, 1], F32, name=f"bT{k}") for k in range(2)]
            a2b = consts.tile([128, MEM_DIM], F32, name="a2b")
            b2b = consts.tile([128, MEM_DIM], F32, name="b2b")

            def rsqrt_dve(dst, src, iters=3, seed="recip", gp=False):
                """dst = 1/sqrt(src), entirely on DVE, keeping Sqrt off the
                ACT engine so the Exp table never gets evicted (each table
                load costs 1.3us). Seed: either a float constant, or
                "recip" = secant fit a*(1/x)+b of sqrt(1/x) over x in
                [60, 400] (max 10% off). Newton steps y*(1.5-0.5*x*y^2)
                converge regardless: 3 iters from 10% -> ~2e-7 rel."""
                shp = list(src.shape)
                eng = nc.gpsimd if gp else nc.vector
                y = work.tile(shp, F32, tag="ny0", name="ny0", bufs=2)
                if seed == "recip":
                    rr = work.tile(shp, F32, tag="nrr", name="nrr", bufs=2)
                    nc.vector.reciprocal(rr, src)
                    eng.tensor_scalar(y, rr, 5.56, 0.0361,
                                      op0=OP.mult, op1=OP.add)
                else:
                    eng.tensor_scalar(y, src, 0.0, float(seed),
                                      op0=OP.mult, op1=OP.add)
                for it in range(iters):
                    t1 = work.tile(shp, F32, tag="nt2", name="nt2", bufs=2)
                    eng.tensor_mul(t1, y, y)
                    eng.tensor_mul(t1, t1, src)
                    eng.tensor_scalar(t1, t1, -0.5, 1.5, op0=OP.mult,
                                      op1=OP.add)
                    out = dst if it == iters - 1 else work.tile(
                        shp, F32, tag=f"ny{it + 1}", name=f"ny{it + 1}", bufs=2)
                    eng.tensor_mul(out, y, t1)
                    y = out

            def stage1a(L, i):
                """lhsT prep for tile i: DMA + transposes + f32r/bf16 splits
                + row-norm rsqrt. Emitted one tile AHEAD of stage1b(i-1)'s
                sims so the ACT queue serves these copies before that tile's
                exp/uts work (otherwise the PE stalls on LDWEIGHTS).
                Small elementwise goes to the idle GpSimd engine."""
                lhsT_f = [
                    work.tile([128, 128], F32, tag=f"lhsTf{k}", name=f"lhsTf{k}", bufs=2)
                    for k in range(2)
                ]
                lhsT_r = [
                    work.tile([128, 128], MMDT, tag=f"lhsTr{k}", name=f"lhsTr{k}", bufs=2)
                    for k in range(2)
                ]
                lhsT_s = [
                    work.tile([128, 128], MMDT, tag=f"lhsTs{k}", name=f"lhsTs{k}", bufs=2)
                    for k in range(2)
                ]
                lhsT_rb = [
                    work.tile([128, 128], BF16, tag=f"lhsTb{k}", name=f"lhsTb{k}", bufs=2)
                    for k in range(2)
                ]
                if L == 1:
                    invn = invn_all[:, i:i + 1]
                    xi = work.tile([128, MEM_DIM], F32, tag="xi", name="xi", bufs=3)
                    nc.sync.dma_start(xi, x_d[i * 128:(i + 1) * 128, :])
                    tpx = psum_tp.tile([128, 256], F32, tag="tp2")
                    for k in range(2):
                        PE.transpose(tpx[:, k * 128:(k + 1) * 128],
                                     xi[:, k * 128:(k + 1) * 128], ident)
                    for k in range(2):
                        nc.scalar.copy(lhsT_f[k], tpx[:, k * 128:(k + 1) * 128])
                else:
                    ns_ps = psum_tp.tile([1, 128], F32, tag="tp2")
                    hsl = work.tile([128, MEM_DIM], F32, tag="h1i", name="h1i", bufs=3)
                    nc.sync.dma_start(hsl, h1_dram[i * 128:(i + 1) * 128, :])
                    sqTs = []
                    tph = psum_tp.tile([128, 256], F32, tag="tp2")
                    for k in range(2):
                        PE.transpose(tph[:, k * 128:(k + 1) * 128],
                                     hsl[:, k * 128:(k + 1) * 128], ident)
                    for k in range(2):
                        # fused BN1 apply + leaky relu at the drain (Prelu
                        # shares the Exp ACT table set; Lrelu doesn't)
                        nc.scalar.activation(
                            lhsT_f[k], tph[:, k * 128:(k + 1) * 128], AF.Prelu,
                            bias=bT[k], scale=aT[k], alpha=LEAKY,
                        )
                        sqT = work.tile([128, 128], F32, tag=f"sqT{k}", name=f"sqT{k}", bufs=2)
                        nc.scalar.activation(sqT, lhsT_f[k], AF.Square)
                        sqTs.append(sqT)
                    for k in range(2):
                        PE.matmul(ns_ps, ones_col, sqTs[k],
                                  start=(k == 0), stop=(k == 1))
                    ns_sb = work.tile([1, 128], F32, tag="ns_sb", name="ns_sb", bufs=2)
                    nc.scalar.copy(ns_sb, ns_ps)
                    nsT = psum_tp.tile([128, 1], F32, tag="tp2")
                    PE.transpose(nsT, ns_sb, one_1x1)
                    invn = work.tile([128, 1], F32, tag="invn", name="invn",
                                     bufs=2)
                    nsS = work.tile([128, 1], F32, tag="nsS", name="nsS", bufs=2)
                    nc.scalar.copy(nsS, nsT)
                    # the rsqrt chain is emitted LATER (after stage1b of the
                    # previous tile) so the DVE queue serves topk first;
                    # invn is only needed at the exp, well past topk.
                for k in range(2):
                    nc.scalar.copy(lhsT_r[k], lhsT_f[k])   # f32r round
                    nc.vector.tensor_copy(lhsT_rb[k], lhsT_f[k])  # bf16 (pass C)
                    rsd = work.tile([128, 128], F32, tag="rsd", name="rsd", bufs=2)
                    nc.vector.tensor_sub(rsd, lhsT_f[k], lhsT_r[k].bitcast(F32))
                    nc.scalar.copy(lhsT_s[k], rsd)         # f32r residual
                return dict(lhsT_r=lhsT_r, lhsT_s=lhsT_s, lhsT_rb=lhsT_rb,
                            invn=invn, nsS=None if L == 1 else nsS)

            def emit_invn(sa):
                if sa is not None and sa.get("nsS") is not None:
                    rsqrt_dve(sa["invn"], sa["nsS"])

            def stage1b(L, i, sa):
                """3-pass sim matmul + top-10 threshold + masked exp
                weights for tile i (lhsT prepped by stage1a)."""
                lhsT_r, lhsT_s = sa["lhsT_r"], sa["lhsT_s"]
                lhsT_rb, invn = sa["lhsT_rb"], sa["invn"]
                # 3-pass f32r sim matmul: r(x)@r(m) + res_x@r(m) + x_b@res_m_b
                s_sb = work.tile([128, MEM_SIZE], F32, tag="s_sb", name="s_sb", bufs=2)
                for c in range(4):
                    ps = psum_sim.tile([128, 512], F32, tag="sim")
                    for k in range(2):
                        PE.matmul(ps, lhsT_r[k],
                                  mnT[L][k][:, c * 512:(c + 1) * 512],
                                  start=(k == 0), stop=False)
                    for k in range(2):
                        PE.matmul(ps, lhsT_s[k],
                                  mnT[L][k][:, c * 512:(c + 1) * 512],
                                  start=False, stop=False)
                    for k in range(2):
                        PE.matmul(ps, lhsT_rb[k],
                                  mnTres[L][k][:, c * 512:(c + 1) * 512],
                                  start=False, stop=(k == 1))
                    nc.scalar.copy(s_sb[:, c * 512:(c + 1) * 512], ps)

                # exact 10th-largest threshold
                m8a = work.tile([128, 8], F32, tag="m8a", name="m8a", bufs=2)
                nc.vector.max(out=m8a, in_=s_sb)
                s_z = work.tile([128, MEM_SIZE], F32, tag="s_z", name="s_z", bufs=2)
                nc.vector.match_replace(out=s_z, in_to_replace=m8a,
                                        in_values=s_sb, imm_value=NEG_BIG)
                m8b = work.tile([128, 8], F32, tag="m8b", name="m8b", bufs=2)
                nc.vector.max(out=m8b, in_=s_z)
                t_ap = m8b[:, K_TOP - 8 - 1:K_TOP - 8]  # 10th largest
                neg_t = work.tile([128, 1], F32, tag="neg_t", name="neg_t", bufs=2)
                nc.vector.tensor_scalar(neg_t, t_ap, invn, -1.0,
                                        op0=OP.mult, op1=OP.mult)

                # shifted exp weights, masked, with sum
                e = work.tile([128, MEM_SIZE], F32 if L == 1 else BF16,
                              tag="e", name="e", bufs=1)
                nc.scalar.activation(e, s_sb, AF.Exp, bias=neg_t, scale=invn)
                if L == 1:
                    U = s_z  # reuse buffer: s_z is dead after m8b
                else:
                    # bf16 weights: DVE rounds on write, so the PE's bf16
                    # fast path (1 cy/row transposes + h2 matmul) is legal
                    U = work.tile([128, MEM_SIZE], BF16, tag="Ub", name="Ub",
                                  bufs=2)
                Z = work.tile([128, 1], F32, tag="Z", name="Z", bufs=2)
                nc.vector.scalar_tensor_tensor(
                    out=U, in0=s_sb, scalar=t_ap, in1=e,
                    op0=OP.is_ge, op1=OP.mult, accum_out=Z,
                )
                rz = work.tile([128, 1], F32, tag="rz", name="rz", bufs=2)
                nc.vector.reciprocal(rz, Z)
                return dict(U=U, rz=rz)

            def stage2(L, i, st, stats_acc, pend):
                """U transposes + h = (U/Z) @ mem + BN batch-stat partials.

                The stats matmuls for THIS tile are deferred: they're pushed
                on `pend` and emitted by the NEXT stage2 call (or the flush),
                so the PE never waits on the dst/sqh drain chain.
                """
                U, rz = st["U"], st["rz"]
                # layer 1 feeds layer-2 sims: must stay fp32-exact.
                # layer 2 only feeds the final output: bf16 is plenty and
                # transposes at 1 cy/row instead of fp32's 2.5.
                ut_dt = F32 if L == 1 else BF16
                tp_ident = ident if L == 1 else ident_b
                # paired transposes -> one [128,256] drain per pair
                uts = []
                for c2 in range(NJ // 2):
                    tp2 = psum_tp.tile([128, 256], ut_dt, tag="tp2")
                    PE.transpose(tp2[:, 0:128],
                                 U[:, (2 * c2) * 128:(2 * c2 + 1) * 128], tp_ident)
                    PE.transpose(tp2[:, 128:256],
                                 U[:, (2 * c2 + 1) * 128:(2 * c2 + 2) * 128], tp_ident)
                    utp = work.tile([128, 256], ut_dt, tag="ut", name="ut",
                                    bufs=NJ // 2 + 2)
                    nc.scalar.copy(utp, tp2)
                    uts.append(utp)
                hp = psum_h_pool.tile([128, MEM_DIM], F32, tag="hp")
                for c in range(NJ):
                    PE.matmul(
                        hp, uts[c // 2][:, (c % 2) * 128:(c % 2 + 1) * 128],
                        mraw_b[L][:, c * MEM_DIM:(c + 1) * MEM_DIM],
                        start=(c == 0), stop=(c == NJ - 1),
                    )
                # drain h (normalized by Z) + square; stats matmuls deferred.
                # h2 is drained + spilled + stat'd in bf16 (halves the tail
                # DMA; BN2 normalizes the rounded values consistently).
                dst = work.tile([128, MEM_DIM], F32 if L == 1 else BF16,
                                tag="h2o", name="h2o", bufs=4)
                nc.scalar.mul(dst, hp, rz)
                h_dram = h1_dram if L == 1 else h2_dram
                nc.sync.dma_start(h_dram[i * 128:(i + 1) * 128, :], dst)
                sqh = work.tile([128, MEM_DIM], F32, tag="sqh", name="sqh", bufs=3)
                nc.vector.tensor_mul(sqh, dst, dst)
                emit_pending_stats(L, stats_acc, pend)
                pend.append((dst, sqh))

            def emit_pending_stats(L, stats_acc, pend):
                ones_l = ones_col if L == 1 else ones_col_b
                while pend:
                    dst, sqh = pend.pop(0)
                    pd = psum_st.tile([1, 512], F32, tag="st")
                    PE.matmul(pd[:, 0:MEM_DIM], ones_l, dst,
                              start=True, stop=True)
                    PE.matmul(pd[:, MEM_DIM:2 * MEM_DIM], ones_col, sqh,
                              start=True, stop=True)
                    nc.vector.tensor_add(stats_acc, stats_acc, pd)

            def layer(L):
                stats_acc = work.tile([1, 512], F32, tag=f"stacc{L}", bufs=1,
                                      name=f"stats_acc{L}")
                nc.vector.memset(stats_acc, 0.0)
                pend = []
                prev = None
                sa = stage1a(L, 0)
                emit_invn(sa)
                for i in range(nt):
                    sa_next = stage1a(L, i + 1) if i + 1 < nt else None
                    st = stage1b(L, i, sa)
                    emit_invn(sa_next)
                    sa = sa_next
                    if prev is not None:
                        stage2(L, i - 1, prev, stats_acc, pend)
                    prev = st
                stage2(L, nt - 1, prev, stats_acc, pend)
                emit_pending_stats(L, stats_acc, pend)
                return stats_acc

            def bn_allreduce(L, stats_acc):
                gamma_sb, beta_sb = gb[L]
                stats_sb = stats_acc
                ar_in = dram.tile([1, 512], F32, name=f"ar_in{L}")
                ar_out = dram.tile([1, 512], F32, addr_space="Shared",
                                   name=f"ar_out{L}")
                nc.sync.dma_start(ar_in, stats_sb)
                nc.gpsimd.collective_compute(
                    "AllReduce", OP.add,
                    replica_groups=[list(range(n_cores))],
                    ins=[ar_in[:]], outs=[ar_out[:]],
                )
                gst = work.tile([1, 512], F32, tag="gst", name="gst", bufs=1)
                nc.sync.dma_start(gst, ar_out)

                ab = work.tile([1, 512], F32, tag="ab", name="ab", bufs=1)
                a_ap, b_ap = ab[:, 0:MEM_DIM], ab[:, MEM_DIM:512]
                mu = work.tile([1, MEM_DIM], F32, tag="mu", name="mu", bufs=1)
                nc.vector.tensor_scalar(mu, gst[:, 0:MEM_DIM], 1.0 / n_total,
                                        None, op0=OP.mult)
                ex2 = work.tile([1, MEM_DIM], F32, tag="ex2", name="ex2", bufs=1)
                nc.vector.tensor_scalar(ex2, gst[:, MEM_DIM:512], 1.0 / n_total,
                                        None, op0=OP.mult)
                musq = work.tile([1, MEM_DIM], F32, tag="musq", name="musq", bufs=1)
                nc.vector.tensor_mul(musq, mu, mu)
                var = work.tile([1, MEM_DIM], F32, tag="var", name="var", bufs=1)
                nc.vector.tensor_sub(var, ex2, musq)
                sd = work.tile([1, MEM_DIM], F32, tag="sd", name="sd", bufs=1)
                nc.scalar.activation(sd, var, AF.Sqrt, bias=epsap)
                isd = work.tile([1, MEM_DIM], F32, tag="isd", name="isd", bufs=1)
                nc.vector.reciprocal(isd, sd)
                nc.vector.tensor_mul(a_ap, gamma_sb, isd)
                mua = work.tile([1, MEM_DIM], F32, tag="mua", name="mua", bufs=1)
                nc.vector.tensor_mul(mua, mu, a_ap)
                nc.vector.tensor_sub(b_ap, beta_sb, mua)

                if L == 1:
                    # per-partition (transposed-layout) affine params
                    for k in range(2):
                        for src, dstp in ((a_ap, aT[k]), (b_ap, bT[k])):
                            tp = psum_tp.tile([128, 1], F32, tag="tp2")
                            PE.transpose(
                                tp, src[:, k * 128:(k + 1) * 128], one_1x1)
                            nc.scalar.copy(dstp, tp)
                else:
                    # broadcast across partitions (row-layout affine)
                    bc = psum_sim.tile([128, 512], F32, tag="sim")
                    PE.matmul(bc, ones_row, ab, start=True, stop=True)
                    nc.scalar.copy(a2b, bc[:, 0:MEM_DIM])
                    nc.scalar.copy(b2b, bc[:, MEM_DIM:512])

            # L1 row-norm prologue: second read of x during the prep
            # window computes every tile's 1/||x|| in one batched chain,
            # so the steady-state DVE queue holds only topk+weights work.
            invn_all = consts.tile([128, nt], F32, name="invn_all")
            xns_all = consts.tile([128, nt], F32, name="xns_all")
            prep_bank(1)
            for i in range(nt):
                xpr = work.tile([128, MEM_DIM], F32, tag="xpr", name="xpr",
                                bufs=3)
                nc.sync.dma_start(xpr, x_d[i * 128:(i + 1) * 128, :])
                xsq = work.tile([128, MEM_DIM], F32, tag="sqs", name="sqs",
                                bufs=2)
                nc.vector.scalar_tensor_tensor(
                    out=xsq, in0=xpr, scalar=0.0, in1=xpr,
                    op0=OP.add, op1=OP.mult, accum_out=xns_all[:, i:i + 1])
            rsqrt_dve(invn_all, xns_all, iters=3, seed="recip")
            s1 = layer(1)
            prep_bank(2)  # fills the AllReduce bubble + L1 tail
            bn_allreduce(1, s1)
            bn_allreduce(2, layer(2))

            # ---- final: BN2 apply + leaky (Prelu) + store out ----
            # process TG row-tiles per wide op: fewer instructions and
            # semaphore round-trips in this PE-less serial tail
            TG = 2
            a2w = consts.tile([128, TG * MEM_DIM], F32, name="a2w")
            b2w = consts.tile([128, TG * MEM_DIM], F32, name="b2w")
            for q in range(TG):
                nc.scalar.copy(a2w[:, q * MEM_DIM:(q + 1) * MEM_DIM], a2b)
                nc.scalar.copy(b2w[:, q * MEM_DIM:(q + 1) * MEM_DIM], b2b)
            for i in range(0, nt, TG):
                hw_t = work.tile([128, TG * MEM_DIM], BF16, tag="h2i",
                                 name="h2i", bufs=3)
                for q in range(TG):
                    nc.sync.dma_start(
                        hw_t[:, q * MEM_DIM:(q + 1) * MEM_DIM],
                        h2_dram[(i + q) * 128:(i + q + 1) * 128, :])
                y = work.tile([128, TG * MEM_DIM], F32, tag="y", name="y", bufs=2)
                nc.vector.tensor_mul(y, hw_t, a2w)
                nc.vector.tensor_add(y, y, b2w)
                nc.scalar.activation(y, y, AF.Prelu, alpha=LEAKY)
                for q in range(TG):
                    nc.sync.dma_start(
                        out_d[(i + q) * 128:(i + q + 1) * 128, :],
                        y[:, q * MEM_DIM:(q + 1) * MEM_DIM])

    nc.compile()
    return nc


_CACHE = {}


def _get_nc(n_cores, rows_per_core, use_f32r=True):
    key = (n_cores, rows_per_core, use_f32r)
    if key not in _CACHE:
        _CACHE[key] = build_nc(n_cores, rows_per_core, use_f32r)
    return _CACHE[key]


def kernel(x, mem1, mem2, gamma1, beta1, gamma2, beta2, _trace=False,
           _use_f32r=True, _n_cores=8):
    n_cores = _n_cores
    n, d = x.shape
    rows_per_core = n // n_cores
    nc = _get_nc(n_cores, rows_per_core, _use_f32r)

    in_maps = []
    for c in range(n_cores):
        in_maps.append({
            "x": np.ascontiguousarray(x[c * rows_per_core:(c + 1) * rows_per_core]),
            "mem1": np.ascontiguousarray(mem1),
            "mem2": np.ascontiguousarray(mem2),
            "gamma1": np.ascontiguousarray(gamma1.reshape(1, -1)),
            "beta1": np.ascontiguousarray(beta1.reshape(1, -1)),
            "gamma2": np.ascontiguousarray(gamma2.reshape(1, -1)),
            "beta2": np.ascontiguousarray(beta2.reshape(1, -1)),
        })
    res = run_bass_kernel_spmd(nc, in_maps, list(range(n_cores)), trace=_trace)
    out = np.concatenate([res.results[c]["out"] for c in range(n_cores)], axis=0)
    if _trace:
        return out, res
    return out
